# revision 1
# baseline (speedup 1.0000x reference)
"""KAN layer (per-edge tiny MLPs) Trainium2 kernel.

Math (per batch b, output o, input i; H=32 hidden):
  h1 = leaky(x[b,i]*W1[o,i,:] + b1[o,i,:])
  z2 = W2[o,i] @ h1 + b2[o,i]           (per-edge [H,H] matmul)
  h2 = leaky(z2)
  edge = W3[o,i]·h2 + b3[o,i]
  out[b,o] = sum_i (bias_w[o,i]*leaky(x[b,i]) + layer_w[o,i]*edge)

Mapping (8 cores, O sharded, 8 output rows per core):
  - x replicated 32x on host -> ACT computes h1 = Lrelu(W1[p]*xrep + b1[p])
    in one pass per (o, i-group of 4), layout [128=(4i x 32h), B].
  - PE (float32r): block-diagonal W2^T [128,128] per (o,g) -> z2 in PSUM;
    folded contractions: st4 = c2 * (layer_w*W3) on h2-ish, st4b = w~2 on h1
    ... actually h2 here is the true leaky, so st4 = layer_w*W3 directly.
  - z2 evac: ACT Lrelu(z2 + b2[p]) or DVE 2-pass leaky (load balance split).
  - All output contractions accumulate into one [8, B] PSUM region:
    st4[og] [128,8] (col o = layer_w*W3 stack), st5 [65,8] carries
    bias_w·leaky(x) + all constants.
"""
import sys

sys.path.insert(0, "/opt/trn_rl_repo")

import numpy as np

_B, _I, _O, _H = 1024, 64, 64, 32
_NCORES = 8
_OLOC = _O // _NCORES  # 8 output nodes per core
_ALPHA = 0.01
_NHALF = 512

# (o,g) blocks whose z2-evac runs on DVE (2-pass leaky) instead of ACT:
# DVE is ~2.4x the per-element cost of ACT here, but ACT also carries all of
# h1 generation, so ~60% of evacs go to DVE to balance the two engines.
def _on_dve(og):
    return og % 5 < 3

_CACHE = {}


def _build_bass():
    import concourse.bacc as bacc
    import concourse.mybir as mybir
    from concourse.tile import TileContext

    f32 = mybir.dt.float32
    f32r = mybir.dt.float32r
    AF = mybir.ActivationFunctionType
    ALU = mybir.AluOpType

    nc = bacc.Bacc("TRN2", target_bir_lowering=False, debug=False)

    xrep_d = nc.declare_dram_parameter("xrep", [2048, _B], f32, isOutput=False)
    xt65_d = nc.declare_dram_parameter("xt65", [65, _B], f32, isOutput=False)
    w1col_d = nc.declare_dram_parameter("w1col", [128, 128], f32, isOutput=False)
    b1col_d = nc.declare_dram_parameter("b1col", [128, 128], f32, isOutput=False)
    b2col_d = nc.declare_dram_parameter("b2col", [128, 128], f32, isOutput=False)
    w2blk_d = nc.declare_dram_parameter("w2blk", [128, 128, 128], f32r, isOutput=False)
    st4_d = nc.declare_dram_parameter("st4", [128, 128 * 8], f32r, isOutput=False)
    st5_d = nc.declare_dram_parameter("st5", [65, 8], f32r, isOutput=False)
    out_d = nc.declare_dram_parameter("out", [8, _B], f32, isOutput=True)

    with TileContext(nc) as tc:
        with tc.tile_pool(name="consts", bufs=1) as cpool, \
             tc.tile_pool(name="w2", bufs=2) as w2pool, \
             tc.tile_pool(name="h1", bufs=5) as h1pool, \
             tc.tile_pool(name="h2", bufs=5) as h2pool, \
             tc.tile_pool(name="a01", bufs=4) as a01pool, \
             tc.tile_pool(name="zps", bufs=3, space="PSUM") as zpool, \
             tc.tile_pool(name="ops", bufs=1, space="PSUM") as opool:

            xrep_t = cpool.tile([128, 16 * _B], f32)
            nc.sync.dma_start(
                out=xrep_t[:].rearrange("p (g n) -> p g n", g=16),
                in_=xrep_d[:].rearrange("(g p) n -> p g n", p=128),
            )
            xt65_t = cpool.tile([65, _B], f32)
            nc.sync.dma_start(out=xt65_t[:], in_=xt65_d[:])
            w1col_t = cpool.tile([128, 128], f32)
            nc.sync.dma_start(out=w1col_t[:], in_=w1col_d[:])
            b1col_t = cpool.tile([128, 128], f32)
            nc.sync.dma_start(out=b1col_t[:], in_=b1col_d[:])
            b2col_t = cpool.tile([128, 128], f32)
            nc.sync.dma_start(out=b2col_t[:], in_=b2col_d[:])
            st4_t = cpool.tile([128, 128 * 8], f32r)
            nc.sync.dma_start(out=st4_t[:], in_=st4_d[:])
            st5_t = cpool.tile([65, 8], f32r)
            nc.sync.dma_start(out=st5_t[:], in_=st5_d[:])

            lxT_t = cpool.tile([65, _B], f32r)
            nc.scalar.activation(lxT_t[:], xt65_t[:], AF.Lrelu,
                                 bias=0.0, scale=1.0, alpha=_ALPHA)

            outp = opool.tile([8, _B], f32)
            # MM5 first: seeds the accumulator (start=True per half/bank)
            for half in range(2):
                sl = slice(half * _NHALF, (half + 1) * _NHALF)
                nc.tensor.matmul(out=outp[:, sl], lhsT=st5_t[:], rhs=lxT_t[:, sl],
                                 start=True, stop=False, skip_group_check=True)

            def emit_mm4(h2_prev, og_prev, last):
                for half in range(2):
                    sl = slice(half * _NHALF, (half + 1) * _NHALF)
                    nc.tensor.matmul(out=outp[:, sl],
                                     lhsT=st4_t[:, og_prev * 8:(og_prev + 1) * 8],
                                     rhs=h2_prev[:, sl], start=False, stop=last,
                                     skip_group_check=True)

            pending = None  # (h2, og) one block behind, so PE never waits on evac
            for o in range(_OLOC):
                w2_t = w2pool.tile([128, 16 * 128], f32r)
                nc.sync.dma_start(
                    out=w2_t[:].rearrange("p (g m) -> p g m", g=16),
                    in_=w2blk_d[o * 16:(o + 1) * 16].rearrange("g p m -> p g m"),
                )
                for g in range(16):
                    og = o * 16 + g
                    h1 = h1pool.tile([128, _B], f32r)
                    nc.scalar.activation(
                        h1[:], xrep_t[:, g * _B:(g + 1) * _B], AF.Lrelu,
                        bias=b1col_t[:, og:og + 1], scale=w1col_t[:, og:og + 1],
                        alpha=_ALPHA)
                    z2 = zpool.tile([128, _B], f32)
                    for half in range(2):
                        sl = slice(half * _NHALF, (half + 1) * _NHALF)
                        nc.tensor.matmul(out=z2[:, sl],
                                         lhsT=w2_t[:, g * 128:(g + 1) * 128],
                                         rhs=h1[:, sl], start=True, stop=True)
                    h2 = h2pool.tile([128, _B], f32r)
                    if _on_dve(og):
                        a01 = a01pool.tile([128, _B], f32)
                        nc.vector.tensor_scalar(
                            out=a01[:], in0=z2[:], scalar1=b2col_t[:, og:og + 1],
                            scalar2=_ALPHA, op0=ALU.add, op1=ALU.mult)
                        nc.vector.scalar_tensor_tensor(
                            out=h2[:], in0=z2[:], scalar=b2col_t[:, og:og + 1],
                            in1=a01[:], op0=ALU.add, op1=ALU.max)
                    else:
                        nc.scalar.activation(h2[:], z2[:], AF.Lrelu,
                                             bias=b2col_t[:, og:og + 1],
                                             scale=1.0, alpha=_ALPHA)
                    if pending is not None:
                        emit_mm4(*pending, last=False)
                    pending = (h2, og)
            emit_mm4(*pending, last=True)

            outs = cpool.tile([8, _B], f32)
            nc.vector.tensor_copy(outs[:], outp[:])
            nc.sync.dma_start(out=out_d[:], in_=outs[:])

    nc.finalize()
    return nc


def _prepare_inputs(x, W1, b1, W2, b2, W3, b3, layer_w, bias_w):
    c1 = (1.0 + _ALPHA) / 2.0
    f = np.float32
    x = np.asarray(x, f)
    xT = np.ascontiguousarray(x.T)                      # [I, B]
    xrep = np.repeat(xT, _H, axis=0)                    # [2048, B]
    xt65 = np.concatenate([xT, np.ones((1, _B), f)], 0)  # [65, B]

    v = (np.asarray(layer_w, f)[:, :, None] * np.asarray(W3, f))  # [O,I,H]
    w2f = np.asarray(W2, f)

    in_maps = []
    for c in range(_NCORES):
        sl = slice(c * _OLOC, (c + 1) * _OLOC)
        W1c, b1c, b2c = W1[sl], b1[sl], b2[sl]          # [8,64,H]
        W2c = w2f[sl]                                   # [8,64,H,H]
        vc = v[sl]
        lwc, bwc, b3c = layer_w[sl], bias_w[sl], b3[sl]

        # [o, g, j, h] -> partition 32j+h, col o*16+g
        def cols(a):  # a [8, 64, 32] -> [128, 128]
            a = np.asarray(a, f).reshape(_OLOC, 16, 4, _H)
            return np.ascontiguousarray(
                a.transpose(2, 3, 0, 1).reshape(128, 128))

        w1col = cols(W1c)
        b1col = cols(b1c)
        b2col = cols(b2c)

        # block-diagonal lhsT: blk[og][32j+h, 32j+k] = W2[o,4g+j,k,h]
        W2t = W2c.transpose(0, 1, 3, 2).reshape(_OLOC, 16, 4, _H, _H)
        w2blk = np.zeros((_OLOC, 16, 128, 128), f)
        for j in range(4):
            w2blk[:, :, 32 * j:32 * j + 32, 32 * j:32 * j + 32] = W2t[:, :, j]
        w2blk = w2blk.reshape(128, 128, 128)

        # st4[og][32j+k, o] = v[o,4g+j,k] ; st4b[og][32j+h, o] = wt2[o,4g+j,h]
        def stack8b(a):
            a = np.asarray(a, f).reshape(_OLOC, 16, 4 * _H)
            out = np.zeros((128, _OLOC * 16, _OLOC), f)
            for o in range(_OLOC):
                for g in range(16):
                    out[:, o * 16 + g, o] = a[o, g]
            return np.ascontiguousarray(out.reshape(128, 128 * _OLOC))

        st4 = stack8b(vc)

        st5 = np.zeros((65, _OLOC), f)
        st5[:_I, :] = np.asarray(bwc, f).T              # bias_w[o,i] at row i
        const = (np.asarray(lwc, f) * np.asarray(b3c, f)).sum(1)
        st5[_I, :] = const

        in_maps.append({
            "xrep": xrep, "xt65": xt65,
            "w1col": w1col, "b1col": b1col, "b2col": b2col,
            "w2blk": w2blk, "st4": st4, "st5": st5,
        })
    return in_maps


def kernel(x, W1, b1, W2, b2, W3, b3, layer_w, bias_w):
    from concourse.bass_utils import run_bass_kernel_spmd

    if "nc" not in _CACHE:
        _CACHE["nc"] = _build_bass()
    nc = _CACHE["nc"]

    in_maps = _prepare_inputs(x, W1, b1, W2, b2, W3, b3, layer_w, bias_w)
    res = run_bass_kernel_spmd(nc, in_maps, list(range(_NCORES))).results

    out = np.empty((_B, _O), np.float32)
    for c in range(_NCORES):
        out[:, c * _OLOC:(c + 1) * _OLOC] = res[c]["out"].T
    return out


if __name__ == "__main__":
    # quick self-check against a numpy reference
    rng = np.random.default_rng(0)
    f = np.float32
    inputs = {
        "x": rng.standard_normal((_B, _I), f),
        "W1": rng.uniform(-1, 1, (_O, _I, _H)).astype(f),
        "b1": rng.uniform(-1, 1, (_O, _I, _H)).astype(f),
        "W2": rng.uniform(-0.2, 0.2, (_O, _I, _H, _H)).astype(f),
        "b2": rng.uniform(-0.2, 0.2, (_O, _I, _H)).astype(f),
        "W3": rng.uniform(-0.2, 0.2, (_O, _I, _H)).astype(f),
        "b3": rng.uniform(-0.2, 0.2, (_O, _I)).astype(f),
        "layer_w": np.ones((_O, _I), f),
        "bias_w": rng.uniform(-0.1, 0.1, (_O, _I)).astype(f),
    }

    def leaky(a):
        return np.where(a >= 0, a, _ALPHA * a)

    def ref(x, W1, b1, W2, b2, W3, b3, layer_w, bias_w):
        h1 = leaky(x[:, None, :, None] * W1 + b1)
        h2 = leaky(np.einsum("boih,oikh->boik", h1, W2) + b2)
        edge = np.einsum("boih,oih->boi", h2, W3) + b3
        edge = bias_w * leaky(x)[:, None, :] + layer_w * edge
        return edge.sum(axis=2)

    expected = ref(**{k: np.asarray(val, np.float64) for k, val in inputs.items()})
    actual = kernel(**inputs)
    err = np.abs(actual - expected).max() / np.abs(expected).max()
    print("rel err:", err)



# revision 2
# speedup vs baseline: 1.0553x; 1.0553x over previous
"""KAN layer (per-edge tiny MLPs) Trainium2 kernel.

Per (b, o, i), H=32:  h1 = leaky(x*W1 + b1); z2 = W2 @ h1;
  h2 = leaky(z2 + b2); edge = W3.h2 + b3;
  out[b,o] = sum_i bias_w*leaky(x) + layer_w*edge.

Mapping (8 cores, O sharded; per core 8 o x 64 i = 512 edges in 128
4-edge blocks, bi = o*16 + g):
  - |v| = |layer_w*W3| folded into W2 columns (leaky(|v|t) = |v|leaky(t));
    signs move into the mm4 weights. b2 pre-scaled likewise.
  - h1: ACT fused Lrelu(scale,bias) 1 instr, or DVE 3-instr bf16 path
    (tensor_scalar 4x + 4x + tensor_tensor max 2x), or Pool 2-instr.
  - mm2: bf16 block-diag [128,128] lhsT -> z2 PSUM.
  - z2-evac: ACT fused Lrelu+bias (1 instr) for most blocks; for the
    rest DVE extracts t=(z2+vb2) to bf16 SBUF (GPSIMD cannot touch
    PSUM on HW) and Pool finishes leaky via one scalar_tensor_tensor.
  - mm4: per-block [128,8] bf16 matmul; for F8 g-pairs, h2 is written
    as x64-scaled fp8-e4m3 pair tiles and contracted 2-blocks-per-
    matmul with DoubleRow at 0.5 cyc/row (weights +-2^-6). The fp8
    fraction is the accuracy/speed knob (err ~ 2.3% * sqrt(frac)).
  - mm5 seeds the [8,B] accumulator with bias_w.leaky(x) + lw.b3.
"""
import sys

sys.path.insert(0, "/opt/trn_rl_repo")

import numpy as np
import ml_dtypes

_B, _I, _O, _H = 1024, 64, 64, 32
_NCORES = 8
_OLOC = _O // _NCORES      # 8 output nodes per core
_NBLK = _OLOC * 16         # 128 blocks per core
_ALPHA = 0.01
_NH = 512                  # psum bank half

# ---- static assignment knobs -------------------------------------------
_F8_GPAIRS = []   # fp8/DoubleRow g-pairs per o (disabled: DR crashed on HW)
_F8_SCALE = 64.0                        # 2^6 shifts h2 into e4m3 normal range
_Z2_DVE_MOD = 8                         # z2-evac on DVE+Pool when bi%MOD==5
# h1 engine cycle (16-long, applied per block index): D=DVE, P=Pool, A=ACT
_H1_CYCLE = "DPDPDPDPDPDPDPDD"

_F8_GS = sorted(g for p in _F8_GPAIRS for g in p)


def _assignments():
    z2_eng = {}   # bi -> 'act' | 'dve'
    h1_eng = {}   # bi -> 'act' | 'dve' | 'pool'
    for bi in range(_NBLK):
        z2_eng[bi] = "act"
        h1_eng[bi] = "act" if (bi % 8 == 3) else "dvp"
    return z2_eng, h1_eng


_CACHE = {}


def _build_bass():
    import concourse.bacc as bacc
    import concourse.mybir as mybir
    from concourse.tile import TileContext

    f32 = mybir.dt.float32
    bf16 = mybir.dt.bfloat16
    fp8 = mybir.dt.float8e4
    AF = mybir.ActivationFunctionType
    ALU = mybir.AluOpType
    PM = mybir.MatmulPerfMode

    z2_eng, h1_eng = _assignments()
    nf8 = len(_F8_GS)
    nbf = 16 - nf8
    ndr = _OLOC * (nf8 // 2)

    nc = bacc.Bacc("TRN2", target_bir_lowering=False, debug=False)

    xt65_d = nc.declare_dram_parameter("xt65", [65, _B], bf16, isOutput=False)
    xrep_d = nc.declare_dram_parameter("xrep", [128, 16 * _B], bf16, isOutput=False)
    w2blk_d = nc.declare_dram_parameter("w2blk", [128, _NBLK * 128], bf16, isOutput=False)
    w1c_d = nc.declare_dram_parameter("w1c", [128, _NBLK], f32, isOutput=False)
    aw1c_d = nc.declare_dram_parameter("aw1c", [128, _NBLK], f32, isOutput=False)
    ab1c_d = nc.declare_dram_parameter("ab1c", [128, _NBLK], f32, isOutput=False)
    b1c_d = nc.declare_dram_parameter("b1c", [128, _NBLK], f32, isOutput=False)
    vb2_d = nc.declare_dram_parameter("vb2", [128, _NBLK], f32, isOutput=False)
    st4b_d = nc.declare_dram_parameter("st4b", [128, _OLOC * nbf * _OLOC], bf16, isOutput=False)
    if ndr:
        st4d_d = nc.declare_dram_parameter("st4d", [128, 2 * ndr * _OLOC], fp8, isOutput=False)
    st5_d = nc.declare_dram_parameter("st5", [65, _OLOC], bf16, isOutput=False)
    out_d = nc.declare_dram_parameter("out", [_OLOC, _B], f32, isOutput=True)

    with TileContext(nc) as tc:
        with tc.tile_pool(name="consts", bufs=1) as cpool, \
             tc.tile_pool(name="h1p", bufs=6) as h1pool, \
             tc.tile_pool(name="h2b", bufs=4) as h2bpool, \
             tc.tile_pool(name="h2p", bufs=2) as h2ppool, \
             tc.tile_pool(name="ut", bufs=4) as upool, \
             tc.tile_pool(name="zps", bufs=3, space="PSUM") as zpool, \
             tc.tile_pool(name="ops", bufs=1, space="PSUM") as opool:

            w1c_t = cpool.tile([128, _NBLK], f32)
            nc.sync.dma_start(out=w1c_t[:], in_=w1c_d[:])
            b1c_t = cpool.tile([128, _NBLK], f32)
            nc.sync.dma_start(out=b1c_t[:], in_=b1c_d[:])
            aw1c_t = cpool.tile([128, _NBLK], f32)
            nc.sync.dma_start(out=aw1c_t[:], in_=aw1c_d[:])
            ab1c_t = cpool.tile([128, _NBLK], f32)
            nc.sync.dma_start(out=ab1c_t[:], in_=ab1c_d[:])
            vb2_t = cpool.tile([128, _NBLK], f32)
            nc.sync.dma_start(out=vb2_t[:], in_=vb2_d[:])
            xt65_t = cpool.tile([65, _B], bf16)
            nc.sync.dma_start(out=xt65_t[:], in_=xt65_d[:])
            st5_t = cpool.tile([65, _OLOC], bf16)
            nc.sync.dma_start(out=st5_t[:], in_=st5_d[:])
            st4b_t = cpool.tile([128, _OLOC * nbf * _OLOC], bf16)
            nc.sync.dma_start(out=st4b_t[:], in_=st4b_d[:])
            if ndr:
                st4d_t = cpool.tile([128, 2, ndr * _OLOC], fp8)
                nc.sync.dma_start(
                    out=st4d_t[:].rearrange("p a n -> p (a n)"),
                    in_=st4d_d[:])
            w2blk_ts = []
            xrep_ts = []
            for q in range(8):
                sx = slice(q * 2 * _B, (q + 1) * 2 * _B)
                xr_t = cpool.tile([128, 2 * _B], bf16, name=f"xr{q}")
                nc.sync.dma_start(out=xr_t[:], in_=xrep_d[:, sx])
                xrep_ts.append(xr_t)
                s = slice(q * 16 * 128, (q + 1) * 16 * 128)
                w2c_t = cpool.tile([128, 16 * 128], bf16, name=f"w2c{q}")
                nc.sync.dma_start(out=w2c_t[:], in_=w2blk_d[:, s])
                w2blk_ts.append(w2c_t)

            # mm5: seed output accumulator with bias_w.leaky(x) + consts
            lxT_t = cpool.tile([65, _B], bf16)
            nc.scalar.activation(lxT_t[:], xt65_t[:], AF.Lrelu,
                                 bias=0.0, scale=1.0, alpha=_ALPHA)
            outp = opool.tile([_OLOC, _B], f32)
            for h in range(2):
                sl = slice(h * _NH, (h + 1) * _NH)
                nc.tensor.matmul(out=outp[:, sl], lhsT=st5_t[:], rhs=lxT_t[:, sl],
                                 start=True, stop=False, skip_group_check=True)

            bf_idx = 0
            dr_idx = 0
            pair_tile = [None]

            for bi in range(_NBLK):
                o, g = divmod(bi, 16)
                f8 = g in _F8_GS

                # ---------- h1 ----------
                h1_t = h1pool.tile([128, _B], bf16)
                he = h1_eng[bi]
                xg = xrep_ts[g // 2][:, (g % 2) * _B:(g % 2 + 1) * _B]
                if he == "act":
                    nc.scalar.activation(
                        h1_t[:], xg, AF.Lrelu,
                        bias=b1c_t[:, bi:bi + 1], scale=w1c_t[:, bi:bi + 1],
                        alpha=_ALPHA)
                else:  # DVE 4-instr bf16 path, single-AP-scalar forms only
                    t_t = upool.tile([128, _B], bf16)
                    nc.vector.tensor_scalar(
                        out=t_t[:], in0=xg, scalar1=w1c_t[:, bi:bi + 1],
                        scalar2=None, op0=ALU.mult)
                    t2_t = upool.tile([128, _B], bf16)
                    nc.vector.tensor_scalar(
                        out=t2_t[:], in0=t_t[:], scalar1=b1c_t[:, bi:bi + 1],
                        scalar2=None, op0=ALU.add)
                    u_t = upool.tile([128, _B], bf16)
                    nc.vector.tensor_scalar(
                        out=u_t[:], in0=t2_t[:], scalar1=_ALPHA,
                        scalar2=None, op0=ALU.mult)
                    nc.vector.tensor_tensor(
                        out=h1_t[:], in0=t2_t[:], in1=u_t[:], op=ALU.max)

                # ---------- mm2 ----------
                z2 = zpool.tile([128, _B], f32)
                for h in range(2):
                    sl = slice(h * _NH, (h + 1) * _NH)
                    nc.tensor.matmul(
                        out=z2[:, sl],
                        lhsT=w2blk_ts[bi // 16][:, (bi % 16) * 128:(bi % 16 + 1) * 128],
                        rhs=h1_t[:, sl], start=True, stop=True)

                # ---------- z2 evac ----------
                if f8:
                    if g % 2 == 0 or pair_tile[0] is None:
                        pair_tile[0] = h2ppool.tile([128, 2, _B], fp8, name="h2pair")
                    h2slot = pair_tile[0][:, g % 2, :]
                else:
                    h2bf = h2bpool.tile([128, _B], bf16)
                    h2slot = h2bf[:]
                if z2_eng[bi] == "act":
                    nc.scalar.activation(h2slot, z2[:], AF.Lrelu,
                                         bias=vb2_t[:, bi:bi + 1],
                                         scale=1.0, alpha=_ALPHA)
                else:  # DVE extract (bias) + Pool leaky finish
                    tt_ = upool.tile([128, _B], bf16)
                    nc.vector.tensor_scalar(
                        out=tt_[:], in0=z2[:], scalar1=vb2_t[:, bi:bi + 1],
                        scalar2=1.0, op0=ALU.add, op1=ALU.mult)
                    nc.gpsimd.scalar_tensor_tensor(
                        out=h2slot, in0=tt_[:], scalar=_ALPHA,
                        in1=tt_[:], op0=ALU.mult, op1=ALU.max)

                # ---------- mm4 ----------
                last = bi == _NBLK - 1
                if f8:
                    if g % 2 == 1:
                        pt = pair_tile[0]
                        for h in range(2):
                            sl = slice(h * _NH, (h + 1) * _NH)
                            nc.tensor.matmul(
                                out=outp[:, sl],
                                lhsT=st4d_t[:, :, dr_idx * _OLOC:(dr_idx + 1) * _OLOC],
                                rhs=pt[:, :, sl],
                                start=False, stop=last and h == 1,
                                perf_mode=PM.DoubleRow,
                                skip_group_check=True)
                        dr_idx += 1
                else:
                    for h in range(2):
                        sl = slice(h * _NH, (h + 1) * _NH)
                        nc.tensor.matmul(
                            out=outp[:, sl],
                            lhsT=st4b_t[:, bf_idx * _OLOC:(bf_idx + 1) * _OLOC],
                            rhs=h2bf[:, sl],
                            start=False, stop=last and h == 1,
                            skip_group_check=True)
                    bf_idx += 1

            outs = cpool.tile([_OLOC, _B], f32)
            for h in range(2):
                sl = slice(h * _NH, (h + 1) * _NH)
                nc.vector.tensor_copy(outs[:, sl], outp[:, sl])
                nc.sync.dma_start(out=out_d[:, sl], in_=outs[:, sl])

    nc.finalize()
    return nc


def _prepare_inputs(x, W1, b1, W2, b2, W3, b3, layer_w, bias_w):
    f = np.float32
    bfd = ml_dtypes.bfloat16
    f8d = ml_dtypes.float8_e4m3fn
    nf8 = len(_F8_GS)

    x = np.asarray(x, f)
    xT = np.ascontiguousarray(x.T)                       # [I, B]
    xt65 = np.concatenate([xT, np.ones((1, _B), f)], 0)  # [65, B]
    # xrep[(32j+h), g*B+b] = x[g*4+j, b]
    xrep = np.repeat(xT.reshape(16, 4, 1, _B), _H, axis=2)   # [16,4,H,B]
    xrep = np.ascontiguousarray(
        xrep.transpose(1, 2, 0, 3).reshape(128, 16 * _B))

    v = np.asarray(layer_w, f)[:, :, None] * np.asarray(W3, f)   # [O,I,H]
    av = np.abs(v)
    sgn = np.where(v >= 0, 1.0, -1.0).astype(f)
    W1 = np.asarray(W1, f); b1 = np.asarray(b1, f)
    W2 = np.asarray(W2, f); b2 = np.asarray(b2, f)
    layer_w = np.asarray(layer_w, f); bias_w = np.asarray(bias_w, f)
    b3 = np.asarray(b3, f)

    in_maps = []
    for c in range(_NCORES):
        osl = slice(c * _OLOC, (c + 1) * _OLOC)
        W1c, b1c, W2c, b2c = W1[osl], b1[osl], W2[osl], b2[osl]
        avc, sgc = av[osl], sgn[osl]
        lwc, bwc, b3c = layer_w[osl], bias_w[osl], b3[osl]

        w2blk = np.zeros((128, _NBLK, 128), f)
        vb2 = np.zeros((128, _NBLK), f)
        w1cc = np.zeros((128, _NBLK), f)
        b1cc = np.zeros((128, _NBLK), f)
        st4b_list, st4d_list = [], []
        pend_dr = None

        for bi in range(_NBLK):
            o, g = divmod(bi, 16)
            f8 = g in _F8_GS
            sc = _F8_SCALE if f8 else 1.0
            edges = [g * 4 + j for j in range(4)]
            for j, i in enumerate(edges):
                w2blk[j * _H:(j + 1) * _H, bi, j * _H:(j + 1) * _H] = \
                    (sc * avc[o, i][:, None] * W2c[o, i]).T
                vb2[j * _H:(j + 1) * _H, bi] = sc * avc[o, i] * b2c[o, i]
                w1cc[j * _H:(j + 1) * _H, bi] = W1c[o, i]
                b1cc[j * _H:(j + 1) * _H, bi] = b1c[o, i]
            col = np.zeros((128, _OLOC), f)
            for j, i in enumerate(edges):
                col[j * _H:(j + 1) * _H, o] = sgc[o, i]
            if f8:
                if g % 2 == 0:
                    pend_dr = col / _F8_SCALE
                else:
                    st4d_list.append((pend_dr, col / _F8_SCALE))
            else:
                st4b_list.append(col)

        st4b = np.concatenate(st4b_list, axis=1)
        ndr = len(st4d_list)
        st4d = np.zeros((128, 2, ndr * _OLOC), f)
        for t, (c0, c1) in enumerate(st4d_list):
            st4d[:, 0, t * _OLOC:(t + 1) * _OLOC] = c0
            st4d[:, 1, t * _OLOC:(t + 1) * _OLOC] = c1

        st5 = np.zeros((65, _OLOC), f)
        st5[:_I, :] = bwc.T
        st5[_I, :] = (lwc * b3c).sum(1)

        m = {
            "xt65": xt65.astype(bfd),
            "xrep": xrep.astype(bfd),
            "w2blk": np.ascontiguousarray(
                w2blk.reshape(128, _NBLK * 128)).astype(bfd),
            "w1c": w1cc, "b1c": b1cc, "vb2": vb2,
            "aw1c": _ALPHA * w1cc, "ab1c": _ALPHA * b1cc,
            "st4b": st4b.astype(bfd),
            "st5": st5.astype(bfd),
        }
        if ndr:
            m["st4d"] = np.ascontiguousarray(
                st4d.reshape(128, 2 * ndr * _OLOC)).astype(f8d)
        in_maps.append(m)
    return in_maps


def kernel(x, W1, b1, W2, b2, W3, b3, layer_w, bias_w):
    from concourse.bass_utils import run_bass_kernel_spmd

    if "nc" not in _CACHE:
        _CACHE["nc"] = _build_bass()
    nc = _CACHE["nc"]

    in_maps = _prepare_inputs(x, W1, b1, W2, b2, W3, b3, layer_w, bias_w)
    res = run_bass_kernel_spmd(nc, in_maps, list(range(_NCORES))).results

    out = np.empty((_B, _O), np.float32)
    for c in range(_NCORES):
        out[:, c * _OLOC:(c + 1) * _OLOC] = res[c]["out"].T
    return out


# revision 3
# speedup vs baseline: 1.0635x; 1.0078x over previous
"""KAN layer (per-edge tiny MLPs) Trainium2 kernel.

Per (b, o, i), H=32:  h1 = leaky(x*W1 + b1); z2 = W2 @ h1;
  h2 = leaky(z2 + b2); edge = W3.h2 + b3;
  out[b,o] = sum_i bias_w*leaky(x) + layer_w*edge.

Mapping (8 cores, O sharded; per core 8 o x 64 i = 512 edges in 128
4-edge blocks, bi = o*16 + g):
  - |v| = |layer_w*W3| folded into W2 columns (leaky(|v|t) = |v|leaky(t));
    signs move into the mm4 weights. b2 pre-scaled likewise.
  - h1: ACT fused Lrelu(scale,bias) 1 instr, or DVE 3-instr bf16 path
    (tensor_scalar 4x + 4x + tensor_tensor max 2x), or Pool 2-instr.
  - mm2: bf16 block-diag [128,128] lhsT -> z2 PSUM.
  - z2-evac: ACT fused Lrelu+bias (1 instr) for most blocks; for the
    rest DVE extracts t=(z2+vb2) to bf16 SBUF (GPSIMD cannot touch
    PSUM on HW) and Pool finishes leaky via one scalar_tensor_tensor.
  - mm4: per-block [128,8] bf16 matmul; for F8 g-pairs, h2 is written
    as x64-scaled fp8-e4m3 pair tiles and contracted 2-blocks-per-
    matmul with DoubleRow at 0.5 cyc/row (weights +-2^-6). The fp8
    fraction is the accuracy/speed knob (err ~ 2.3% * sqrt(frac)).
  - mm5 seeds the [8,B] accumulator with bias_w.leaky(x) + lw.b3.
"""
import sys

sys.path.insert(0, "/opt/trn_rl_repo")

import numpy as np
import ml_dtypes

_B, _I, _O, _H = 1024, 64, 64, 32
_NCORES = 8
_OLOC = _O // _NCORES      # 8 output nodes per core
_NBLK = _OLOC * 16         # 128 blocks per core
_ALPHA = 0.01
_NH = 512                  # psum bank half

# ---- static assignment knobs -------------------------------------------
_F8_GPAIRS = []   # fp8/DoubleRow g-pairs per o (disabled: DR crashed on HW)
_F8_SCALE = 64.0                        # 2^6 shifts h2 into e4m3 normal range
_Z2_DVE_MOD = 8                         # z2-evac on DVE+Pool when bi%MOD==5
# h1 engine cycle (16-long, applied per block index): D=DVE, P=Pool, A=ACT
_H1_CYCLE = "DPDPDPDPDPDPDPDD"

_F8_GS = sorted(g for p in _F8_GPAIRS for g in p)


def _assignments():
    z2_eng = {}   # bi -> 'act' | 'dve'
    h1_eng = {}   # bi -> 'act' | 'dve' | 'pool'
    for bi in range(_NBLK):
        z2_eng[bi] = "act"
        h1_eng[bi] = "act" if (bi % 32 in (1, 5, 9, 13, 17, 21, 25)) else "dvp"
    return z2_eng, h1_eng


_CACHE = {}


def _build_bass():
    import concourse.bacc as bacc
    import concourse.mybir as mybir
    from concourse.tile import TileContext

    f32 = mybir.dt.float32
    bf16 = mybir.dt.bfloat16
    fp8 = mybir.dt.float8e4
    AF = mybir.ActivationFunctionType
    ALU = mybir.AluOpType
    PM = mybir.MatmulPerfMode

    z2_eng, h1_eng = _assignments()
    nf8 = len(_F8_GS)
    nbf = 16 - nf8
    ndr = _OLOC * (nf8 // 2)

    nc = bacc.Bacc("TRN2", target_bir_lowering=False, debug=False)

    xt65_d = nc.declare_dram_parameter("xt65", [65, _B], bf16, isOutput=False)
    xrep_d = nc.declare_dram_parameter("xrep", [128, 16 * _B], bf16, isOutput=False)
    w2blk_d = nc.declare_dram_parameter("w2blk", [128, _NBLK * 128], bf16, isOutput=False)
    w1c_d = nc.declare_dram_parameter("w1c", [128, _NBLK], f32, isOutput=False)
    aw1c_d = nc.declare_dram_parameter("aw1c", [128, _NBLK], f32, isOutput=False)
    ab1c_d = nc.declare_dram_parameter("ab1c", [128, _NBLK], f32, isOutput=False)
    b1c_d = nc.declare_dram_parameter("b1c", [128, _NBLK], f32, isOutput=False)
    vb2_d = nc.declare_dram_parameter("vb2", [128, _NBLK], f32, isOutput=False)
    st4b_d = nc.declare_dram_parameter("st4b", [128, _OLOC * nbf * _OLOC], bf16, isOutput=False)
    if ndr:
        st4d_d = nc.declare_dram_parameter("st4d", [128, 2 * ndr * _OLOC], fp8, isOutput=False)
    st5_d = nc.declare_dram_parameter("st5", [65, _OLOC], bf16, isOutput=False)
    out_d = nc.declare_dram_parameter("out", [_OLOC, _B], f32, isOutput=True)

    with TileContext(nc) as tc:
        with tc.tile_pool(name="consts", bufs=1) as cpool, \
             tc.tile_pool(name="h1p", bufs=6) as h1pool, \
             tc.tile_pool(name="h2b", bufs=4) as h2bpool, \
             tc.tile_pool(name="h2p", bufs=2) as h2ppool, \
             tc.tile_pool(name="ut", bufs=4) as upool, \
             tc.tile_pool(name="zps", bufs=3, space="PSUM") as zpool, \
             tc.tile_pool(name="ops", bufs=1, space="PSUM") as opool:

            w1c_t = cpool.tile([128, _NBLK], f32)
            nc.sync.dma_start(out=w1c_t[:], in_=w1c_d[:])
            b1c_t = cpool.tile([128, _NBLK], f32)
            nc.sync.dma_start(out=b1c_t[:], in_=b1c_d[:])
            aw1c_t = cpool.tile([128, _NBLK], f32)
            nc.sync.dma_start(out=aw1c_t[:], in_=aw1c_d[:])
            ab1c_t = cpool.tile([128, _NBLK], f32)
            nc.sync.dma_start(out=ab1c_t[:], in_=ab1c_d[:])
            vb2_t = cpool.tile([128, _NBLK], f32)
            nc.sync.dma_start(out=vb2_t[:], in_=vb2_d[:])
            xt65_t = cpool.tile([65, _B], bf16)
            nc.sync.dma_start(out=xt65_t[:], in_=xt65_d[:])
            st5_t = cpool.tile([65, _OLOC], bf16)
            nc.sync.dma_start(out=st5_t[:], in_=st5_d[:])
            st4b_t = cpool.tile([128, _OLOC * nbf * _OLOC], bf16)
            nc.sync.dma_start(out=st4b_t[:], in_=st4b_d[:])
            if ndr:
                st4d_t = cpool.tile([128, 2, ndr * _OLOC], fp8)
                nc.sync.dma_start(
                    out=st4d_t[:].rearrange("p a n -> p (a n)"),
                    in_=st4d_d[:])
            w2blk_ts = []
            xrep_ts = []
            for q in range(8):
                sx = slice(q * 2 * _B, (q + 1) * 2 * _B)
                xr_t = cpool.tile([128, 2 * _B], bf16, name=f"xr{q}")
                nc.sync.dma_start(out=xr_t[:], in_=xrep_d[:, sx])
                xrep_ts.append(xr_t)
                s = slice(q * 16 * 128, (q + 1) * 16 * 128)
                w2c_t = cpool.tile([128, 16 * 128], bf16, name=f"w2c{q}")
                nc.sync.dma_start(out=w2c_t[:], in_=w2blk_d[:, s])
                w2blk_ts.append(w2c_t)

            # mm5: seed output accumulator with bias_w.leaky(x) + consts
            lxT_t = cpool.tile([65, _B], bf16)
            nc.scalar.activation(lxT_t[:], xt65_t[:], AF.Lrelu,
                                 bias=0.0, scale=1.0, alpha=_ALPHA)
            outp = opool.tile([_OLOC, _B], f32)
            for h in range(2):
                sl = slice(h * _NH, (h + 1) * _NH)
                nc.tensor.matmul(out=outp[:, sl], lhsT=st5_t[:], rhs=lxT_t[:, sl],
                                 start=True, stop=False, skip_group_check=True)

            bf_idx = 0
            dr_idx = 0
            pair_tile = [None]

            for bi in range(_NBLK):
                o, g = divmod(bi, 16)
                f8 = g in _F8_GS

                # ---------- h1 ----------
                h1_t = h1pool.tile([128, _B], bf16)
                he = h1_eng[bi]
                xg = xrep_ts[g // 2][:, (g % 2) * _B:(g % 2 + 1) * _B]
                if he == "act":
                    nc.scalar.activation(
                        h1_t[:], xg, AF.Lrelu,
                        bias=b1c_t[:, bi:bi + 1], scale=w1c_t[:, bi:bi + 1],
                        alpha=_ALPHA)
                else:  # DVE 4-instr bf16 path, single-AP-scalar forms only
                    t_t = upool.tile([128, _B], bf16)
                    nc.vector.tensor_scalar(
                        out=t_t[:], in0=xg, scalar1=w1c_t[:, bi:bi + 1],
                        scalar2=None, op0=ALU.mult)
                    t2_t = upool.tile([128, _B], bf16)
                    nc.vector.tensor_scalar(
                        out=t2_t[:], in0=t_t[:], scalar1=b1c_t[:, bi:bi + 1],
                        scalar2=None, op0=ALU.add)
                    u_t = upool.tile([128, _B], bf16)
                    nc.vector.tensor_scalar(
                        out=u_t[:], in0=t2_t[:], scalar1=_ALPHA,
                        scalar2=None, op0=ALU.mult)
                    nc.vector.tensor_tensor(
                        out=h1_t[:], in0=t2_t[:], in1=u_t[:], op=ALU.max)

                # ---------- mm2 ----------
                z2 = zpool.tile([128, _B], f32)
                for h in range(2):
                    sl = slice(h * _NH, (h + 1) * _NH)
                    nc.tensor.matmul(
                        out=z2[:, sl],
                        lhsT=w2blk_ts[bi // 16][:, (bi % 16) * 128:(bi % 16 + 1) * 128],
                        rhs=h1_t[:, sl], start=True, stop=True)

                # ---------- z2 evac ----------
                if f8:
                    if g % 2 == 0 or pair_tile[0] is None:
                        pair_tile[0] = h2ppool.tile([128, 2, _B], fp8, name="h2pair")
                    h2slot = pair_tile[0][:, g % 2, :]
                else:
                    h2bf = h2bpool.tile([128, _B], bf16)
                    h2slot = h2bf[:]
                if z2_eng[bi] == "act":
                    nc.scalar.activation(h2slot, z2[:], AF.Lrelu,
                                         bias=vb2_t[:, bi:bi + 1],
                                         scale=1.0, alpha=_ALPHA)
                else:  # DVE extract (bias) + Pool leaky finish
                    tt_ = upool.tile([128, _B], bf16)
                    nc.vector.tensor_scalar(
                        out=tt_[:], in0=z2[:], scalar1=vb2_t[:, bi:bi + 1],
                        scalar2=1.0, op0=ALU.add, op1=ALU.mult)
                    nc.gpsimd.scalar_tensor_tensor(
                        out=h2slot, in0=tt_[:], scalar=_ALPHA,
                        in1=tt_[:], op0=ALU.mult, op1=ALU.max)

                # ---------- mm4 ----------
                last = bi == _NBLK - 1
                if f8:
                    if g % 2 == 1:
                        pt = pair_tile[0]
                        for h in range(2):
                            sl = slice(h * _NH, (h + 1) * _NH)
                            nc.tensor.matmul(
                                out=outp[:, sl],
                                lhsT=st4d_t[:, :, dr_idx * _OLOC:(dr_idx + 1) * _OLOC],
                                rhs=pt[:, :, sl],
                                start=False, stop=last and h == 1,
                                perf_mode=PM.DoubleRow,
                                skip_group_check=True)
                        dr_idx += 1
                else:
                    for h in range(2):
                        sl = slice(h * _NH, (h + 1) * _NH)
                        nc.tensor.matmul(
                            out=outp[:, sl],
                            lhsT=st4b_t[:, bf_idx * _OLOC:(bf_idx + 1) * _OLOC],
                            rhs=h2bf[:, sl],
                            start=False, stop=last and h == 1,
                            skip_group_check=True)
                    bf_idx += 1

            outs = cpool.tile([_OLOC, _B], f32)
            for h in range(2):
                sl = slice(h * _NH, (h + 1) * _NH)
                nc.vector.tensor_copy(outs[:, sl], outp[:, sl])
                nc.sync.dma_start(out=out_d[:, sl], in_=outs[:, sl])

    nc.finalize()
    return nc


def _prepare_inputs(x, W1, b1, W2, b2, W3, b3, layer_w, bias_w):
    f = np.float32
    bfd = ml_dtypes.bfloat16
    f8d = ml_dtypes.float8_e4m3fn
    nf8 = len(_F8_GS)

    x = np.asarray(x, f)
    xT = np.ascontiguousarray(x.T)                       # [I, B]
    xt65 = np.concatenate([xT, np.ones((1, _B), f)], 0)  # [65, B]
    # xrep[(32j+h), g*B+b] = x[g*4+j, b]
    xrep = np.repeat(xT.reshape(16, 4, 1, _B), _H, axis=2)   # [16,4,H,B]
    xrep = np.ascontiguousarray(
        xrep.transpose(1, 2, 0, 3).reshape(128, 16 * _B))

    v = np.asarray(layer_w, f)[:, :, None] * np.asarray(W3, f)   # [O,I,H]
    av = np.abs(v)
    sgn = np.where(v >= 0, 1.0, -1.0).astype(f)
    W1 = np.asarray(W1, f); b1 = np.asarray(b1, f)
    W2 = np.asarray(W2, f); b2 = np.asarray(b2, f)
    layer_w = np.asarray(layer_w, f); bias_w = np.asarray(bias_w, f)
    b3 = np.asarray(b3, f)

    in_maps = []
    for c in range(_NCORES):
        osl = slice(c * _OLOC, (c + 1) * _OLOC)
        W1c, b1c, W2c, b2c = W1[osl], b1[osl], W2[osl], b2[osl]
        avc, sgc = av[osl], sgn[osl]
        lwc, bwc, b3c = layer_w[osl], bias_w[osl], b3[osl]

        w2blk = np.zeros((128, _NBLK, 128), f)
        vb2 = np.zeros((128, _NBLK), f)
        w1cc = np.zeros((128, _NBLK), f)
        b1cc = np.zeros((128, _NBLK), f)
        st4b_list, st4d_list = [], []
        pend_dr = None

        for bi in range(_NBLK):
            o, g = divmod(bi, 16)
            f8 = g in _F8_GS
            sc = _F8_SCALE if f8 else 1.0
            edges = [g * 4 + j for j in range(4)]
            for j, i in enumerate(edges):
                w2blk[j * _H:(j + 1) * _H, bi, j * _H:(j + 1) * _H] = \
                    (sc * avc[o, i][:, None] * W2c[o, i]).T
                vb2[j * _H:(j + 1) * _H, bi] = sc * avc[o, i] * b2c[o, i]
                w1cc[j * _H:(j + 1) * _H, bi] = W1c[o, i]
                b1cc[j * _H:(j + 1) * _H, bi] = b1c[o, i]
            col = np.zeros((128, _OLOC), f)
            for j, i in enumerate(edges):
                col[j * _H:(j + 1) * _H, o] = sgc[o, i]
            if f8:
                if g % 2 == 0:
                    pend_dr = col / _F8_SCALE
                else:
                    st4d_list.append((pend_dr, col / _F8_SCALE))
            else:
                st4b_list.append(col)

        st4b = np.concatenate(st4b_list, axis=1)
        ndr = len(st4d_list)
        st4d = np.zeros((128, 2, ndr * _OLOC), f)
        for t, (c0, c1) in enumerate(st4d_list):
            st4d[:, 0, t * _OLOC:(t + 1) * _OLOC] = c0
            st4d[:, 1, t * _OLOC:(t + 1) * _OLOC] = c1

        st5 = np.zeros((65, _OLOC), f)
        st5[:_I, :] = bwc.T
        st5[_I, :] = (lwc * b3c).sum(1)

        m = {
            "xt65": xt65.astype(bfd),
            "xrep": xrep.astype(bfd),
            "w2blk": np.ascontiguousarray(
                w2blk.reshape(128, _NBLK * 128)).astype(bfd),
            "w1c": w1cc, "b1c": b1cc, "vb2": vb2,
            "aw1c": _ALPHA * w1cc, "ab1c": _ALPHA * b1cc,
            "st4b": st4b.astype(bfd),
            "st5": st5.astype(bfd),
        }
        if ndr:
            m["st4d"] = np.ascontiguousarray(
                st4d.reshape(128, 2 * ndr * _OLOC)).astype(f8d)
        in_maps.append(m)
    return in_maps


def kernel(x, W1, b1, W2, b2, W3, b3, layer_w, bias_w):
    from concourse.bass_utils import run_bass_kernel_spmd

    if "nc" not in _CACHE:
        _CACHE["nc"] = _build_bass()
    nc = _CACHE["nc"]

    in_maps = _prepare_inputs(x, W1, b1, W2, b2, W3, b3, layer_w, bias_w)
    res = run_bass_kernel_spmd(nc, in_maps, list(range(_NCORES))).results

    out = np.empty((_B, _O), np.float32)
    for c in range(_NCORES):
        out[:, c * _OLOC:(c + 1) * _OLOC] = res[c]["out"].T
    return out


# revision 4
# speedup vs baseline: 1.0812x; 1.0166x over previous
"""KAN layer (per-edge tiny MLPs) Trainium2 kernel.

Per (b, o, i), H=32:  h1 = leaky(x*W1 + b1); z2 = W2 @ h1;
  h2 = leaky(z2 + b2); edge = W3.h2 + b3;
  out[b,o] = sum_i bias_w*leaky(x) + layer_w*edge.

Mapping (8 cores, O sharded; per core 8 o x 64 i = 512 edges in 128
4-edge blocks, bi = o*16 + g):
  - |v| = |layer_w*W3| folded into W2 columns (leaky(|v|t) = |v|leaky(t));
    signs move into the mm4 weights. b2 pre-scaled likewise.
  - h1: ACT fused Lrelu(scale,bias) 1 instr, or DVE 3-instr bf16 path
    (tensor_scalar 4x + 4x + tensor_tensor max 2x), or Pool 2-instr.
  - mm2: bf16 block-diag [128,128] lhsT -> z2 PSUM.
  - z2-evac: ACT fused Lrelu+bias (1 instr) for most blocks; for the
    rest DVE extracts t=(z2+vb2) to bf16 SBUF (GPSIMD cannot touch
    PSUM on HW) and Pool finishes leaky via one scalar_tensor_tensor.
  - mm4: per-block [128,8] bf16 matmul; for F8 g-pairs, h2 is written
    as x64-scaled fp8-e4m3 pair tiles and contracted 2-blocks-per-
    matmul with DoubleRow at 0.5 cyc/row (weights +-2^-6). The fp8
    fraction is the accuracy/speed knob (err ~ 2.3% * sqrt(frac)).
  - mm5 seeds the [8,B] accumulator with bias_w.leaky(x) + lw.b3.
"""
import sys

sys.path.insert(0, "/opt/trn_rl_repo")

import numpy as np
import ml_dtypes

_B, _I, _O, _H = 1024, 64, 64, 32
_NCORES = 8
_OLOC = _O // _NCORES      # 8 output nodes per core
_NBLK = _OLOC * 16         # 128 blocks per core
_ALPHA = 0.01
_NH = 512                  # psum bank half

# ---- static assignment knobs -------------------------------------------
_F8_GPAIRS = []   # fp8/DoubleRow g-pairs per o (disabled: DR crashed on HW)
_F8_SCALE = 64.0                        # 2^6 shifts h2 into e4m3 normal range
_Z2_DVE_MOD = 8                         # z2-evac on DVE+Pool when bi%MOD==5
# h1 engine cycle (16-long, applied per block index): D=DVE, P=Pool, A=ACT
_H1_CYCLE = "DPDPDPDPDPDPDPDD"

_F8_GS = sorted(g for p in _F8_GPAIRS for g in p)


def _assignments():
    z2_eng = {}   # bi -> 'act' | 'dve'
    h1_eng = {}   # bi -> 'act' | 'dve' | 'pool'
    for bi in range(_NBLK):
        z2_eng[bi] = "act"
        h1_eng[bi] = "act" if (bi % 32 in (1, 5, 9, 13, 17, 21, 25)) else "dvp"
    return z2_eng, h1_eng


_CACHE = {}


def _build_bass():
    import concourse.bacc as bacc
    import concourse.mybir as mybir
    from concourse.tile import TileContext

    f32 = mybir.dt.float32
    bf16 = mybir.dt.bfloat16
    fp8 = mybir.dt.float8e4
    AF = mybir.ActivationFunctionType
    ALU = mybir.AluOpType
    PM = mybir.MatmulPerfMode

    z2_eng, h1_eng = _assignments()
    nf8 = len(_F8_GS)
    nbf = 16 - nf8
    ndr = _OLOC * (nf8 // 2)

    nc = bacc.Bacc("TRN2", target_bir_lowering=False, debug=False)

    xt65_d = nc.declare_dram_parameter("xt65", [65, _B], bf16, isOutput=False)
    xrep_d = nc.declare_dram_parameter("xrep", [128, 16 * _B], bf16, isOutput=False)
    w2blk_d = nc.declare_dram_parameter("w2blk", [128, _NBLK * 128], bf16, isOutput=False)
    w1c_d = nc.declare_dram_parameter("w1c", [128, _NBLK], f32, isOutput=False)
    aw1c_d = nc.declare_dram_parameter("aw1c", [128, _NBLK], f32, isOutput=False)
    ab1c_d = nc.declare_dram_parameter("ab1c", [128, _NBLK], f32, isOutput=False)
    b1c_d = nc.declare_dram_parameter("b1c", [128, _NBLK], f32, isOutput=False)
    vb2_d = nc.declare_dram_parameter("vb2", [128, _NBLK], f32, isOutput=False)
    st4b_d = nc.declare_dram_parameter("st4b", [128, _OLOC * nbf * _OLOC], bf16, isOutput=False)
    if ndr:
        st4d_d = nc.declare_dram_parameter("st4d", [128, 2 * ndr * _OLOC], fp8, isOutput=False)
    st5_d = nc.declare_dram_parameter("st5", [65, _OLOC], bf16, isOutput=False)
    out_d = nc.declare_dram_parameter("out", [_OLOC, _B], f32, isOutput=True)

    with TileContext(nc) as tc:
        with tc.tile_pool(name="consts", bufs=1) as cpool, \
             tc.tile_pool(name="h1p", bufs=6) as h1pool, \
             tc.tile_pool(name="h2b", bufs=4) as h2bpool, \
             tc.tile_pool(name="h2p", bufs=2) as h2ppool, \
             tc.tile_pool(name="ut", bufs=4) as upool, \
             tc.tile_pool(name="zps", bufs=3, space="PSUM") as zpool, \
             tc.tile_pool(name="ops", bufs=1, space="PSUM") as opool:

            w1c_t = cpool.tile([128, _NBLK], f32)
            nc.sync.dma_start(out=w1c_t[:], in_=w1c_d[:])
            b1c_t = cpool.tile([128, _NBLK], f32)
            nc.sync.dma_start(out=b1c_t[:], in_=b1c_d[:])
            w2blk_ts = []
            xrep_ts = []
            for q in range(2):
                sx = slice(q * 2 * _B, (q + 1) * 2 * _B)
                xr_t = cpool.tile([128, 2 * _B], bf16, name=f"xr{q}")
                nc.sync.dma_start(out=xr_t[:], in_=xrep_d[:, sx])
                xrep_ts.append(xr_t)
                s = slice(q * 16 * 128, (q + 1) * 16 * 128)
                w2c_t = cpool.tile([128, 16 * 128], bf16, name=f"w2c{q}")
                nc.sync.dma_start(out=w2c_t[:], in_=w2blk_d[:, s])
                w2blk_ts.append(w2c_t)
            vb2_t = cpool.tile([128, _NBLK], f32)
            nc.sync.dma_start(out=vb2_t[:], in_=vb2_d[:])
            st4b_t = cpool.tile([128, _OLOC * nbf * _OLOC], bf16)
            nc.sync.dma_start(out=st4b_t[:], in_=st4b_d[:])
            xt65_t = cpool.tile([65, _B], bf16)
            nc.sync.dma_start(out=xt65_t[:], in_=xt65_d[:])
            st5_t = cpool.tile([65, _OLOC], bf16)
            nc.sync.dma_start(out=st5_t[:], in_=st5_d[:])
            aw1c_t = cpool.tile([128, _NBLK], f32)
            nc.sync.dma_start(out=aw1c_t[:], in_=aw1c_d[:])
            ab1c_t = cpool.tile([128, _NBLK], f32)
            nc.sync.dma_start(out=ab1c_t[:], in_=ab1c_d[:])
            if ndr:
                st4d_t = cpool.tile([128, 2, ndr * _OLOC], fp8)
                nc.sync.dma_start(
                    out=st4d_t[:].rearrange("p a n -> p (a n)"),
                    in_=st4d_d[:])
            for q in range(2, 8):
                sx = slice(q * 2 * _B, (q + 1) * 2 * _B)
                xr_t = cpool.tile([128, 2 * _B], bf16, name=f"xr{q}")
                nc.sync.dma_start(out=xr_t[:], in_=xrep_d[:, sx])
                xrep_ts.append(xr_t)
                s = slice(q * 16 * 128, (q + 1) * 16 * 128)
                w2c_t = cpool.tile([128, 16 * 128], bf16, name=f"w2c{q}")
                nc.sync.dma_start(out=w2c_t[:], in_=w2blk_d[:, s])
                w2blk_ts.append(w2c_t)

            # mm5: seed output accumulator with bias_w.leaky(x) + consts
            lxT_t = cpool.tile([65, _B], bf16)
            nc.scalar.activation(lxT_t[:], xt65_t[:], AF.Lrelu,
                                 bias=0.0, scale=1.0, alpha=_ALPHA)
            outp = opool.tile([_OLOC, _B], f32)
            for h in range(2):
                sl = slice(h * _NH, (h + 1) * _NH)
                nc.tensor.matmul(out=outp[:, sl], lhsT=st5_t[:], rhs=lxT_t[:, sl],
                                 start=True, stop=False, skip_group_check=True)

            bf_idx = 0
            dr_idx = 0
            pair_tile = [None]

            for bi in range(_NBLK):
                o, g = divmod(bi, 16)
                f8 = g in _F8_GS

                # ---------- h1 ----------
                h1_t = h1pool.tile([128, _B], bf16)
                he = h1_eng[bi]
                xg = xrep_ts[g // 2][:, (g % 2) * _B:(g % 2 + 1) * _B]
                if he == "act":
                    nc.scalar.activation(
                        h1_t[:], xg, AF.Lrelu,
                        bias=b1c_t[:, bi:bi + 1], scale=w1c_t[:, bi:bi + 1],
                        alpha=_ALPHA)
                else:  # DVE 4-instr bf16 path, single-AP-scalar forms only
                    t_t = upool.tile([128, _B], bf16)
                    nc.vector.tensor_scalar(
                        out=t_t[:], in0=xg, scalar1=w1c_t[:, bi:bi + 1],
                        scalar2=None, op0=ALU.mult)
                    t2_t = upool.tile([128, _B], bf16)
                    nc.vector.tensor_scalar(
                        out=t2_t[:], in0=t_t[:], scalar1=b1c_t[:, bi:bi + 1],
                        scalar2=None, op0=ALU.add)
                    u_t = upool.tile([128, _B], bf16)
                    nc.vector.tensor_scalar(
                        out=u_t[:], in0=t_t[:], scalar1=b1c_t[:, bi:bi + 1],
                        scalar2=_ALPHA, op0=ALU.add, op1=ALU.mult)
                    nc.vector.tensor_tensor(
                        out=h1_t[:], in0=t2_t[:], in1=u_t[:], op=ALU.max)

                # ---------- mm2 ----------
                z2 = zpool.tile([128, _B], f32)
                for h in range(2):
                    sl = slice(h * _NH, (h + 1) * _NH)
                    nc.tensor.matmul(
                        out=z2[:, sl],
                        lhsT=w2blk_ts[bi // 16][:, (bi % 16) * 128:(bi % 16 + 1) * 128],
                        rhs=h1_t[:, sl], start=True, stop=True)

                # ---------- z2 evac ----------
                if f8:
                    if g % 2 == 0 or pair_tile[0] is None:
                        pair_tile[0] = h2ppool.tile([128, 2, _B], fp8, name="h2pair")
                    h2slot = pair_tile[0][:, g % 2, :]
                else:
                    h2bf = h2bpool.tile([128, _B], bf16)
                    h2slot = h2bf[:]
                if z2_eng[bi] == "act":
                    nc.scalar.activation(h2slot, z2[:], AF.Lrelu,
                                         bias=vb2_t[:, bi:bi + 1],
                                         scale=1.0, alpha=_ALPHA)
                else:  # DVE extract (bias) + Pool leaky finish
                    tt_ = upool.tile([128, _B], bf16)
                    nc.vector.tensor_scalar(
                        out=tt_[:], in0=z2[:], scalar1=vb2_t[:, bi:bi + 1],
                        scalar2=1.0, op0=ALU.add, op1=ALU.mult)
                    nc.gpsimd.scalar_tensor_tensor(
                        out=h2slot, in0=tt_[:], scalar=_ALPHA,
                        in1=tt_[:], op0=ALU.mult, op1=ALU.max)

                # ---------- mm4 ----------
                last = bi == _NBLK - 1
                if f8:
                    if g % 2 == 1:
                        pt = pair_tile[0]
                        for h in range(2):
                            sl = slice(h * _NH, (h + 1) * _NH)
                            nc.tensor.matmul(
                                out=outp[:, sl],
                                lhsT=st4d_t[:, :, dr_idx * _OLOC:(dr_idx + 1) * _OLOC],
                                rhs=pt[:, :, sl],
                                start=False, stop=last and h == 1,
                                perf_mode=PM.DoubleRow,
                                skip_group_check=True)
                        dr_idx += 1
                else:
                    for h in range(2):
                        sl = slice(h * _NH, (h + 1) * _NH)
                        nc.tensor.matmul(
                            out=outp[:, sl],
                            lhsT=st4b_t[:, bf_idx * _OLOC:(bf_idx + 1) * _OLOC],
                            rhs=h2bf[:, sl],
                            start=False, stop=last and h == 1,
                            skip_group_check=True)
                    bf_idx += 1

            outs = cpool.tile([_OLOC, _B], f32)
            for h in range(2):
                sl = slice(h * _NH, (h + 1) * _NH)
                nc.vector.tensor_copy(outs[:, sl], outp[:, sl])
                nc.sync.dma_start(out=out_d[:, sl], in_=outs[:, sl])

    nc.finalize()
    return nc


def _prepare_inputs(x, W1, b1, W2, b2, W3, b3, layer_w, bias_w):
    f = np.float32
    bfd = ml_dtypes.bfloat16
    f8d = ml_dtypes.float8_e4m3fn
    nf8 = len(_F8_GS)

    x = np.asarray(x, f)
    xT = np.ascontiguousarray(x.T)                       # [I, B]
    xt65 = np.concatenate([xT, np.ones((1, _B), f)], 0)  # [65, B]
    # xrep[(32j+h), g*B+b] = x[g*4+j, b]
    xrep = np.repeat(xT.reshape(16, 4, 1, _B), _H, axis=2)   # [16,4,H,B]
    xrep = np.ascontiguousarray(
        xrep.transpose(1, 2, 0, 3).reshape(128, 16 * _B))

    v = np.asarray(layer_w, f)[:, :, None] * np.asarray(W3, f)   # [O,I,H]
    av = np.abs(v)
    sgn = np.where(v >= 0, 1.0, -1.0).astype(f)
    W1 = np.asarray(W1, f); b1 = np.asarray(b1, f)
    W2 = np.asarray(W2, f); b2 = np.asarray(b2, f)
    layer_w = np.asarray(layer_w, f); bias_w = np.asarray(bias_w, f)
    b3 = np.asarray(b3, f)

    in_maps = []
    for c in range(_NCORES):
        osl = slice(c * _OLOC, (c + 1) * _OLOC)
        W1c, b1c, W2c, b2c = W1[osl], b1[osl], W2[osl], b2[osl]
        avc, sgc = av[osl], sgn[osl]
        lwc, bwc, b3c = layer_w[osl], bias_w[osl], b3[osl]

        w2blk = np.zeros((128, _NBLK, 128), f)
        vb2 = np.zeros((128, _NBLK), f)
        w1cc = np.zeros((128, _NBLK), f)
        b1cc = np.zeros((128, _NBLK), f)
        st4b_list, st4d_list = [], []
        pend_dr = None

        for bi in range(_NBLK):
            o, g = divmod(bi, 16)
            f8 = g in _F8_GS
            sc = _F8_SCALE if f8 else 1.0
            edges = [g * 4 + j for j in range(4)]
            for j, i in enumerate(edges):
                w2blk[j * _H:(j + 1) * _H, bi, j * _H:(j + 1) * _H] = \
                    (sc * avc[o, i][:, None] * W2c[o, i]).T
                vb2[j * _H:(j + 1) * _H, bi] = sc * avc[o, i] * b2c[o, i]
                w1cc[j * _H:(j + 1) * _H, bi] = W1c[o, i]
                b1cc[j * _H:(j + 1) * _H, bi] = b1c[o, i]
            col = np.zeros((128, _OLOC), f)
            for j, i in enumerate(edges):
                col[j * _H:(j + 1) * _H, o] = sgc[o, i]
            if f8:
                if g % 2 == 0:
                    pend_dr = col / _F8_SCALE
                else:
                    st4d_list.append((pend_dr, col / _F8_SCALE))
            else:
                st4b_list.append(col)

        st4b = np.concatenate(st4b_list, axis=1)
        ndr = len(st4d_list)
        st4d = np.zeros((128, 2, ndr * _OLOC), f)
        for t, (c0, c1) in enumerate(st4d_list):
            st4d[:, 0, t * _OLOC:(t + 1) * _OLOC] = c0
            st4d[:, 1, t * _OLOC:(t + 1) * _OLOC] = c1

        st5 = np.zeros((65, _OLOC), f)
        st5[:_I, :] = bwc.T
        st5[_I, :] = (lwc * b3c).sum(1)

        m = {
            "xt65": xt65.astype(bfd),
            "xrep": xrep.astype(bfd),
            "w2blk": np.ascontiguousarray(
                w2blk.reshape(128, _NBLK * 128)).astype(bfd),
            "w1c": w1cc, "b1c": b1cc, "vb2": vb2,
            "aw1c": _ALPHA * w1cc, "ab1c": _ALPHA * b1cc,
            "st4b": st4b.astype(bfd),
            "st5": st5.astype(bfd),
        }
        if ndr:
            m["st4d"] = np.ascontiguousarray(
                st4d.reshape(128, 2 * ndr * _OLOC)).astype(f8d)
        in_maps.append(m)
    return in_maps


def kernel(x, W1, b1, W2, b2, W3, b3, layer_w, bias_w):
    from concourse.bass_utils import run_bass_kernel_spmd

    if "nc" not in _CACHE:
        _CACHE["nc"] = _build_bass()
    nc = _CACHE["nc"]

    in_maps = _prepare_inputs(x, W1, b1, W2, b2, W3, b3, layer_w, bias_w)
    res = run_bass_kernel_spmd(nc, in_maps, list(range(_NCORES))).results

    out = np.empty((_B, _O), np.float32)
    for c in range(_NCORES):
        out[:, c * _OLOC:(c + 1) * _OLOC] = res[c]["out"].T
    return out


# revision 5
# speedup vs baseline: 1.1144x; 1.0308x over previous
"""KAN layer (per-edge tiny MLPs) Trainium2 kernel.

Per (b, o, i), H=32:  h1 = leaky(x*W1 + b1); z2 = W2 @ h1;
  h2 = leaky(z2 + b2); edge = W3.h2 + b3;
  out[b,o] = sum_i bias_w*leaky(x) + layer_w*edge.

Mapping (8 cores, O sharded; per core 8 o x 64 i = 512 edges in 128
4-edge blocks, bi = o*16 + g):
  - |v| = |layer_w*W3| folded into W2 columns (leaky(|v|t) = |v|leaky(t));
    signs move into the mm4 weights. b2 pre-scaled likewise.
  - h1: ACT fused Lrelu(scale,bias) 1 instr, or DVE 3-instr bf16 path
    (tensor_scalar 4x + 4x + tensor_tensor max 2x), or Pool 2-instr.
  - mm2: bf16 block-diag [128,128] lhsT -> z2 PSUM.
  - z2-evac: ACT fused Lrelu+bias (1 instr) for most blocks; for the
    rest DVE extracts t=(z2+vb2) to bf16 SBUF (GPSIMD cannot touch
    PSUM on HW) and Pool finishes leaky via one scalar_tensor_tensor.
  - mm4: per-block [128,8] bf16 matmul; for F8 g-pairs, h2 is written
    as x64-scaled fp8-e4m3 pair tiles and contracted 2-blocks-per-
    matmul with DoubleRow at 0.5 cyc/row (weights +-2^-6). The fp8
    fraction is the accuracy/speed knob (err ~ 2.3% * sqrt(frac)).
  - mm5 seeds the [8,B] accumulator with bias_w.leaky(x) + lw.b3.
"""
import sys

sys.path.insert(0, "/opt/trn_rl_repo")

import numpy as np
import ml_dtypes

_B, _I, _O, _H = 1024, 64, 64, 32
_NCORES = 8
_OLOC = _O // _NCORES      # 8 output nodes per core
_NBLK = _OLOC * 16         # 128 blocks per core
_ALPHA = 0.01
_NH = 512                  # psum bank half

# ---- static assignment knobs -------------------------------------------
_F8_GPAIRS = []   # fp8/DoubleRow g-pairs per o (disabled: DR crashed on HW)
_F8_SCALE = 64.0                        # 2^6 shifts h2 into e4m3 normal range
_Z2_DVE_MOD = 8                         # z2-evac on DVE+Pool when bi%MOD==5
# h1 engine cycle (16-long, applied per block index): D=DVE, P=Pool, A=ACT
_H1_CYCLE = "DPDPDPDPDPDPDPDD"

_F8_GS = sorted(g for p in _F8_GPAIRS for g in p)


def _assignments():
    z2_eng = {}   # bi -> 'act' | 'dve'
    h1_eng = {}   # bi -> 'act' | 'dve' | 'pool'
    for bi in range(_NBLK):
        z2_eng[bi] = "act"
        h1_eng[bi] = "act" if (bi % 64 in (1, 5, 9, 13, 17, 21, 25,
                                           33, 37, 41, 45, 49, 53)) else "dvp"
    return z2_eng, h1_eng


_CACHE = {}


def _build_bass():
    import concourse.bacc as bacc
    import concourse.mybir as mybir
    from concourse.tile import TileContext

    f32 = mybir.dt.float32
    bf16 = mybir.dt.bfloat16
    fp8 = mybir.dt.float8e4
    AF = mybir.ActivationFunctionType
    ALU = mybir.AluOpType
    PM = mybir.MatmulPerfMode

    z2_eng, h1_eng = _assignments()
    nf8 = len(_F8_GS)
    nbf = 16 - nf8
    ndr = _OLOC * (nf8 // 2)

    nc = bacc.Bacc("TRN2", target_bir_lowering=False, debug=False)

    xt65_d = nc.declare_dram_parameter("xt65", [65, _B], bf16, isOutput=False)
    xrep_d = nc.declare_dram_parameter("xrep", [128, 16 * _B], bf16, isOutput=False)
    w2blk_d = nc.declare_dram_parameter("w2blk", [128, _NBLK * 128], bf16, isOutput=False)
    w1c_d = nc.declare_dram_parameter("w1c", [128, _NBLK], f32, isOutput=False)
    aw1c_d = nc.declare_dram_parameter("aw1c", [128, _NBLK], f32, isOutput=False)
    ab1c_d = nc.declare_dram_parameter("ab1c", [128, _NBLK], f32, isOutput=False)
    b1c_d = nc.declare_dram_parameter("b1c", [128, _NBLK], f32, isOutput=False)
    vb2_d = nc.declare_dram_parameter("vb2", [128, _NBLK], f32, isOutput=False)
    st4b_d = nc.declare_dram_parameter("st4b", [128, _OLOC * nbf * _OLOC], bf16, isOutput=False)
    if ndr:
        st4d_d = nc.declare_dram_parameter("st4d", [128, 2 * ndr * _OLOC], fp8, isOutput=False)
    st5_d = nc.declare_dram_parameter("st5", [65, _OLOC], bf16, isOutput=False)
    out_d = nc.declare_dram_parameter("out", [_OLOC, _B], f32, isOutput=True)

    with TileContext(nc) as tc:
        with tc.tile_pool(name="consts", bufs=1) as cpool, \
             tc.tile_pool(name="h1p", bufs=6) as h1pool, \
             tc.tile_pool(name="h2b", bufs=4) as h2bpool, \
             tc.tile_pool(name="h2p", bufs=2) as h2ppool, \
             tc.tile_pool(name="ut", bufs=4) as upool, \
             tc.tile_pool(name="zps", bufs=3, space="PSUM") as zpool, \
             tc.tile_pool(name="ops", bufs=1, space="PSUM") as opool:

            w1c_t = cpool.tile([128, _NBLK], f32)
            nc.sync.dma_start(out=w1c_t[:], in_=w1c_d[:])
            b1c_t = cpool.tile([128, _NBLK], f32)
            nc.sync.dma_start(out=b1c_t[:], in_=b1c_d[:])
            xt65_t = cpool.tile([65, _B], bf16)
            nc.sync.dma_start(out=xt65_t[:], in_=xt65_d[:])
            st5_t = cpool.tile([65, _OLOC], bf16)
            nc.sync.dma_start(out=st5_t[:], in_=st5_d[:])
            w2blk_ts = []
            xrep_ts = []
            for q in range(2):
                sx = slice(q * 2 * _B, (q + 1) * 2 * _B)
                xr_t = cpool.tile([128, 2 * _B], bf16, name=f"xr{q}")
                nc.sync.dma_start(out=xr_t[:], in_=xrep_d[:, sx])
                xrep_ts.append(xr_t)
                s = slice(q * 16 * 128, (q + 1) * 16 * 128)
                w2c_t = cpool.tile([128, 16 * 128], bf16, name=f"w2c{q}")
                nc.sync.dma_start(out=w2c_t[:], in_=w2blk_d[:, s])
                w2blk_ts.append(w2c_t)
            vb2_t = cpool.tile([128, _NBLK], f32)
            nc.sync.dma_start(out=vb2_t[:], in_=vb2_d[:])
            st4b_t = cpool.tile([128, _OLOC * nbf * _OLOC], bf16)
            nc.sync.dma_start(out=st4b_t[:], in_=st4b_d[:])
            aw1c_t = cpool.tile([128, _NBLK], f32)
            nc.sync.dma_start(out=aw1c_t[:], in_=aw1c_d[:])
            ab1c_t = cpool.tile([128, _NBLK], f32)
            nc.sync.dma_start(out=ab1c_t[:], in_=ab1c_d[:])
            if ndr:
                st4d_t = cpool.tile([128, 2, ndr * _OLOC], fp8)
                nc.sync.dma_start(
                    out=st4d_t[:].rearrange("p a n -> p (a n)"),
                    in_=st4d_d[:])
            for q in range(2, 8):
                sx = slice(q * 2 * _B, (q + 1) * 2 * _B)
                xr_t = cpool.tile([128, 2 * _B], bf16, name=f"xr{q}")
                nc.sync.dma_start(out=xr_t[:], in_=xrep_d[:, sx])
                xrep_ts.append(xr_t)
                s = slice(q * 16 * 128, (q + 1) * 16 * 128)
                w2c_t = cpool.tile([128, 16 * 128], bf16, name=f"w2c{q}")
                nc.sync.dma_start(out=w2c_t[:], in_=w2blk_d[:, s])
                w2blk_ts.append(w2c_t)

            # mm5: seed output accumulator with bias_w.leaky(x) + consts
            lxT_t = cpool.tile([65, _B], bf16)
            nc.scalar.activation(lxT_t[:], xt65_t[:], AF.Lrelu,
                                 bias=0.0, scale=1.0, alpha=_ALPHA)
            outp = opool.tile([_OLOC, _B], f32)
            for h in range(2):
                sl = slice(h * _NH, (h + 1) * _NH)
                nc.tensor.matmul(out=outp[:, sl], lhsT=st5_t[:], rhs=lxT_t[:, sl],
                                 start=True, stop=False, skip_group_check=True)

            bf_idx = 0
            dr_idx = 0
            pair_tile = [None]

            for bi in range(_NBLK):
                o, g = divmod(bi, 16)
                f8 = g in _F8_GS

                # ---------- h1 ----------
                h1_t = h1pool.tile([128, _B], bf16)
                he = h1_eng[bi]
                xg = xrep_ts[g // 2][:, (g % 2) * _B:(g % 2 + 1) * _B]
                if he == "act":
                    nc.scalar.activation(
                        h1_t[:], xg, AF.Lrelu,
                        bias=b1c_t[:, bi:bi + 1], scale=w1c_t[:, bi:bi + 1],
                        alpha=_ALPHA)
                else:  # DVE 4-instr bf16 path, single-AP-scalar forms only
                    t_t = upool.tile([128, _B], bf16)
                    nc.vector.tensor_scalar(
                        out=t_t[:], in0=xg, scalar1=w1c_t[:, bi:bi + 1],
                        scalar2=None, op0=ALU.mult)
                    t2_t = upool.tile([128, _B], bf16)
                    nc.vector.tensor_scalar(
                        out=t2_t[:], in0=t_t[:], scalar1=b1c_t[:, bi:bi + 1],
                        scalar2=None, op0=ALU.add)
                    u_t = upool.tile([128, _B], bf16)
                    nc.vector.tensor_scalar(
                        out=u_t[:], in0=t_t[:], scalar1=b1c_t[:, bi:bi + 1],
                        scalar2=_ALPHA, op0=ALU.add, op1=ALU.mult)
                    nc.vector.tensor_tensor(
                        out=h1_t[:], in0=t2_t[:], in1=u_t[:], op=ALU.max)

                # ---------- mm2 ----------
                z2 = zpool.tile([128, _B], f32)
                for h in range(2):
                    sl = slice(h * _NH, (h + 1) * _NH)
                    nc.tensor.matmul(
                        out=z2[:, sl],
                        lhsT=w2blk_ts[bi // 16][:, (bi % 16) * 128:(bi % 16 + 1) * 128],
                        rhs=h1_t[:, sl], start=True, stop=True)

                # ---------- z2 evac ----------
                if f8:
                    if g % 2 == 0 or pair_tile[0] is None:
                        pair_tile[0] = h2ppool.tile([128, 2, _B], fp8, name="h2pair")
                    h2slot = pair_tile[0][:, g % 2, :]
                else:
                    h2bf = h2bpool.tile([128, _B], bf16)
                    h2slot = h2bf[:]
                if z2_eng[bi] == "act":
                    nc.scalar.activation(h2slot, z2[:], AF.Lrelu,
                                         bias=vb2_t[:, bi:bi + 1],
                                         scale=1.0, alpha=_ALPHA)
                else:  # DVE extract (bias) + Pool leaky finish
                    tt_ = upool.tile([128, _B], bf16)
                    nc.vector.tensor_scalar(
                        out=tt_[:], in0=z2[:], scalar1=vb2_t[:, bi:bi + 1],
                        scalar2=1.0, op0=ALU.add, op1=ALU.mult)
                    nc.gpsimd.scalar_tensor_tensor(
                        out=h2slot, in0=tt_[:], scalar=_ALPHA,
                        in1=tt_[:], op0=ALU.mult, op1=ALU.max)

                # ---------- mm4 ----------
                last = bi == _NBLK - 1
                if f8:
                    if g % 2 == 1:
                        pt = pair_tile[0]
                        for h in range(2):
                            sl = slice(h * _NH, (h + 1) * _NH)
                            nc.tensor.matmul(
                                out=outp[:, sl],
                                lhsT=st4d_t[:, :, dr_idx * _OLOC:(dr_idx + 1) * _OLOC],
                                rhs=pt[:, :, sl],
                                start=False, stop=last and h == 1,
                                perf_mode=PM.DoubleRow,
                                skip_group_check=True)
                        dr_idx += 1
                else:
                    for h in range(2):
                        sl = slice(h * _NH, (h + 1) * _NH)
                        nc.tensor.matmul(
                            out=outp[:, sl],
                            lhsT=st4b_t[:, bf_idx * _OLOC:(bf_idx + 1) * _OLOC],
                            rhs=h2bf[:, sl],
                            start=False, stop=last and h == 1,
                            skip_group_check=True)
                    bf_idx += 1

            outs = cpool.tile([_OLOC, _B], f32)
            for h in range(2):
                sl = slice(h * _NH, (h + 1) * _NH)
                nc.vector.tensor_copy(outs[:, sl], outp[:, sl])
                nc.sync.dma_start(out=out_d[:, sl], in_=outs[:, sl])

    nc.finalize()
    return nc


def _prepare_inputs(x, W1, b1, W2, b2, W3, b3, layer_w, bias_w):
    f = np.float32
    bfd = ml_dtypes.bfloat16
    f8d = ml_dtypes.float8_e4m3fn
    nf8 = len(_F8_GS)

    x = np.asarray(x, f)
    xT = np.ascontiguousarray(x.T)                       # [I, B]
    xt65 = np.concatenate([xT, np.ones((1, _B), f)], 0)  # [65, B]
    # xrep[(32j+h), g*B+b] = x[g*4+j, b]
    xrep = np.repeat(xT.reshape(16, 4, 1, _B), _H, axis=2)   # [16,4,H,B]
    xrep = np.ascontiguousarray(
        xrep.transpose(1, 2, 0, 3).reshape(128, 16 * _B))

    v = np.asarray(layer_w, f)[:, :, None] * np.asarray(W3, f)   # [O,I,H]
    av = np.abs(v)
    sgn = np.where(v >= 0, 1.0, -1.0).astype(f)
    W1 = np.asarray(W1, f); b1 = np.asarray(b1, f)
    W2 = np.asarray(W2, f); b2 = np.asarray(b2, f)
    layer_w = np.asarray(layer_w, f); bias_w = np.asarray(bias_w, f)
    b3 = np.asarray(b3, f)

    in_maps = []
    for c in range(_NCORES):
        osl = slice(c * _OLOC, (c + 1) * _OLOC)
        W1c, b1c, W2c, b2c = W1[osl], b1[osl], W2[osl], b2[osl]
        avc, sgc = av[osl], sgn[osl]
        lwc, bwc, b3c = layer_w[osl], bias_w[osl], b3[osl]

        w2blk = np.zeros((128, _NBLK, 128), f)
        vb2 = np.zeros((128, _NBLK), f)
        w1cc = np.zeros((128, _NBLK), f)
        b1cc = np.zeros((128, _NBLK), f)
        st4b_list, st4d_list = [], []
        pend_dr = None

        for bi in range(_NBLK):
            o, g = divmod(bi, 16)
            f8 = g in _F8_GS
            sc = _F8_SCALE if f8 else 1.0
            edges = [g * 4 + j for j in range(4)]
            for j, i in enumerate(edges):
                w2blk[j * _H:(j + 1) * _H, bi, j * _H:(j + 1) * _H] = \
                    (sc * avc[o, i][:, None] * W2c[o, i]).T
                vb2[j * _H:(j + 1) * _H, bi] = sc * avc[o, i] * b2c[o, i]
                w1cc[j * _H:(j + 1) * _H, bi] = W1c[o, i]
                b1cc[j * _H:(j + 1) * _H, bi] = b1c[o, i]
            col = np.zeros((128, _OLOC), f)
            for j, i in enumerate(edges):
                col[j * _H:(j + 1) * _H, o] = sgc[o, i]
            if f8:
                if g % 2 == 0:
                    pend_dr = col / _F8_SCALE
                else:
                    st4d_list.append((pend_dr, col / _F8_SCALE))
            else:
                st4b_list.append(col)

        st4b = np.concatenate(st4b_list, axis=1)
        ndr = len(st4d_list)
        st4d = np.zeros((128, 2, ndr * _OLOC), f)
        for t, (c0, c1) in enumerate(st4d_list):
            st4d[:, 0, t * _OLOC:(t + 1) * _OLOC] = c0
            st4d[:, 1, t * _OLOC:(t + 1) * _OLOC] = c1

        st5 = np.zeros((65, _OLOC), f)
        st5[:_I, :] = bwc.T
        st5[_I, :] = (lwc * b3c).sum(1)

        m = {
            "xt65": xt65.astype(bfd),
            "xrep": xrep.astype(bfd),
            "w2blk": np.ascontiguousarray(
                w2blk.reshape(128, _NBLK * 128)).astype(bfd),
            "w1c": w1cc, "b1c": b1cc, "vb2": vb2,
            "aw1c": _ALPHA * w1cc, "ab1c": _ALPHA * b1cc,
            "st4b": st4b.astype(bfd),
            "st5": st5.astype(bfd),
        }
        if ndr:
            m["st4d"] = np.ascontiguousarray(
                st4d.reshape(128, 2 * ndr * _OLOC)).astype(f8d)
        in_maps.append(m)
    return in_maps


def kernel(x, W1, b1, W2, b2, W3, b3, layer_w, bias_w):
    from concourse.bass_utils import run_bass_kernel_spmd

    if "nc" not in _CACHE:
        _CACHE["nc"] = _build_bass()
    nc = _CACHE["nc"]

    in_maps = _prepare_inputs(x, W1, b1, W2, b2, W3, b3, layer_w, bias_w)
    res = run_bass_kernel_spmd(nc, in_maps, list(range(_NCORES))).results

    out = np.empty((_B, _O), np.float32)
    for c in range(_NCORES):
        out[:, c * _OLOC:(c + 1) * _OLOC] = res[c]["out"].T
    return out


# revision 6
# speedup vs baseline: 1.1606x; 1.0414x over previous
"""KAN layer (per-edge tiny MLPs) Trainium2 kernel.

Per (b, o, i), H=32:  h1 = leaky(x*W1 + b1); z2 = W2 @ h1;
  h2 = leaky(z2 + b2); edge = W3.h2 + b3;
  out[b,o] = sum_i bias_w*leaky(x) + layer_w*edge.

Mapping (8 cores, O sharded; per core 8 o x 64 i = 512 edges in 128
4-edge blocks, bi = o*16 + g):
  - |v| = |layer_w*W3| folded into W2 columns (leaky(|v|t) = |v|leaky(t));
    signs move into the mm4 weights. b2 pre-scaled likewise.
  - h1: ACT fused Lrelu(scale,bias) 1 instr, or DVE 3-instr bf16 path
    (tensor_scalar 4x + 4x + tensor_tensor max 2x), or Pool 2-instr.
  - mm2: bf16 block-diag [128,128] lhsT -> z2 PSUM.
  - z2-evac: ACT fused Lrelu+bias (1 instr) for most blocks; for the
    rest DVE extracts t=(z2+vb2) to bf16 SBUF (GPSIMD cannot touch
    PSUM on HW) and Pool finishes leaky via one scalar_tensor_tensor.
  - mm4: per-block [128,8] bf16 matmul; for F8 g-pairs, h2 is written
    as x64-scaled fp8-e4m3 pair tiles and contracted 2-blocks-per-
    matmul with DoubleRow at 0.5 cyc/row (weights +-2^-6). The fp8
    fraction is the accuracy/speed knob (err ~ 2.3% * sqrt(frac)).
  - mm5 seeds the [8,B] accumulator with bias_w.leaky(x) + lw.b3.
"""
import sys

sys.path.insert(0, "/opt/trn_rl_repo")

import numpy as np
import ml_dtypes

_B, _I, _O, _H = 1024, 64, 64, 32
_NCORES = 8
_OLOC = _O // _NCORES      # 8 output nodes per core
_NBLK = _OLOC * 16         # 128 blocks per core
_ALPHA = 0.01
_NH = 512                  # psum bank half

# ---- static assignment knobs -------------------------------------------
_F8_GPAIRS = []   # fp8/DoubleRow g-pairs per o (disabled: DR crashed on HW)
_F8_SCALE = 64.0                        # 2^6 shifts h2 into e4m3 normal range
_Z2_DVE_MOD = 8                         # z2-evac on DVE+Pool when bi%MOD==5
# h1 engine cycle (16-long, applied per block index): D=DVE, P=Pool, A=ACT
_H1_CYCLE = "DPDPDPDPDPDPDPDD"

_F8_GS = sorted(g for p in _F8_GPAIRS for g in p)


def _assignments():
    z2_eng = {}   # bi -> 'act' | 'dve'
    h1_eng = {}   # bi -> 'act' | 'dve' | 'pool'
    for bi in range(_NBLK):
        z2_eng[bi] = "act"
        h1_eng[bi] = "act" if (bi % 64 in (1, 9, 17, 25, 33, 41, 49)) else "dvp"
    return z2_eng, h1_eng


_CACHE = {}


def _build_bass():
    import concourse.bacc as bacc
    import concourse.mybir as mybir
    from concourse.tile import TileContext

    f32 = mybir.dt.float32
    bf16 = mybir.dt.bfloat16
    fp8 = mybir.dt.float8e4
    AF = mybir.ActivationFunctionType
    ALU = mybir.AluOpType
    PM = mybir.MatmulPerfMode

    z2_eng, h1_eng = _assignments()
    nf8 = len(_F8_GS)
    nbf = 16 - nf8
    ndr = _OLOC * (nf8 // 2)

    nc = bacc.Bacc("TRN2", target_bir_lowering=False, debug=False)

    xt65_d = nc.declare_dram_parameter("xt65", [65, _B], bf16, isOutput=False)
    xrep_d = nc.declare_dram_parameter("xrep", [128, 16 * _B], bf16, isOutput=False)
    w2blk_d = nc.declare_dram_parameter("w2blk", [128, _NBLK * 128], bf16, isOutput=False)
    w1c_d = nc.declare_dram_parameter("w1c", [128, _NBLK], f32, isOutput=False)
    aw1c_d = nc.declare_dram_parameter("aw1c", [128, _NBLK], f32, isOutput=False)
    ab1c_d = nc.declare_dram_parameter("ab1c", [128, _NBLK], f32, isOutput=False)
    b1c_d = nc.declare_dram_parameter("b1c", [128, _NBLK], f32, isOutput=False)
    vb2_d = nc.declare_dram_parameter("vb2", [128, _NBLK], f32, isOutput=False)
    st4b_d = nc.declare_dram_parameter("st4b", [128, _OLOC * nbf * _OLOC], bf16, isOutput=False)
    if ndr:
        st4d_d = nc.declare_dram_parameter("st4d", [128, 2 * ndr * _OLOC], fp8, isOutput=False)
    st5_d = nc.declare_dram_parameter("st5", [65, _OLOC], bf16, isOutput=False)
    out_d = nc.declare_dram_parameter("out", [_OLOC, _B], f32, isOutput=True)

    with TileContext(nc) as tc:
        with tc.tile_pool(name="consts", bufs=1) as cpool, \
             tc.tile_pool(name="h1p", bufs=6) as h1pool, \
             tc.tile_pool(name="h2b", bufs=4) as h2bpool, \
             tc.tile_pool(name="h2p", bufs=2) as h2ppool, \
             tc.tile_pool(name="ut", bufs=4) as upool, \
             tc.tile_pool(name="zps", bufs=3, space="PSUM") as zpool, \
             tc.tile_pool(name="ops", bufs=1, space="PSUM") as opool:

            w1c_t = cpool.tile([128, _NBLK], f32)
            nc.sync.dma_start(out=w1c_t[:], in_=w1c_d[:])
            b1c_t = cpool.tile([128, _NBLK], f32)
            nc.sync.dma_start(out=b1c_t[:], in_=b1c_d[:])
            xt65_t = cpool.tile([65, _B], bf16)
            nc.sync.dma_start(out=xt65_t[:], in_=xt65_d[:])
            st5_t = cpool.tile([65, _OLOC], bf16)
            nc.sync.dma_start(out=st5_t[:], in_=st5_d[:])
            w2blk_ts = []
            xrep_ts = []
            for q in range(2):
                sx = slice(q * 2 * _B, (q + 1) * 2 * _B)
                xr_t = cpool.tile([128, 2 * _B], bf16, name=f"xr{q}")
                nc.sync.dma_start(out=xr_t[:], in_=xrep_d[:, sx])
                xrep_ts.append(xr_t)
                s = slice(q * 16 * 128, (q + 1) * 16 * 128)
                w2c_t = cpool.tile([128, 16 * 128], bf16, name=f"w2c{q}")
                nc.sync.dma_start(out=w2c_t[:], in_=w2blk_d[:, s])
                w2blk_ts.append(w2c_t)
            vb2_t = cpool.tile([128, _NBLK], f32)
            nc.sync.dma_start(out=vb2_t[:], in_=vb2_d[:])
            st4b_t = cpool.tile([128, _OLOC * nbf * _OLOC], bf16)
            nc.sync.dma_start(out=st4b_t[:], in_=st4b_d[:])
            aw1c_t = cpool.tile([128, _NBLK], f32)
            nc.sync.dma_start(out=aw1c_t[:], in_=aw1c_d[:])
            ab1c_t = cpool.tile([128, _NBLK], f32)
            nc.sync.dma_start(out=ab1c_t[:], in_=ab1c_d[:])
            if ndr:
                st4d_t = cpool.tile([128, 2, ndr * _OLOC], fp8)
                nc.sync.dma_start(
                    out=st4d_t[:].rearrange("p a n -> p (a n)"),
                    in_=st4d_d[:])
            for q in range(2, 8):
                sx = slice(q * 2 * _B, (q + 1) * 2 * _B)
                xr_t = cpool.tile([128, 2 * _B], bf16, name=f"xr{q}")
                nc.sync.dma_start(out=xr_t[:], in_=xrep_d[:, sx])
                xrep_ts.append(xr_t)
                s = slice(q * 16 * 128, (q + 1) * 16 * 128)
                w2c_t = cpool.tile([128, 16 * 128], bf16, name=f"w2c{q}")
                nc.sync.dma_start(out=w2c_t[:], in_=w2blk_d[:, s])
                w2blk_ts.append(w2c_t)

            # mm5: seed output accumulator with bias_w.leaky(x) + consts
            lxT_t = cpool.tile([65, _B], bf16)
            u65_t = cpool.tile([65, _B], bf16)
            nc.vector.tensor_scalar(out=u65_t[:], in0=xt65_t[:],
                                    scalar1=_ALPHA, scalar2=None, op0=ALU.mult)
            nc.vector.tensor_tensor(out=lxT_t[:], in0=xt65_t[:],
                                    in1=u65_t[:], op=ALU.max)
            outp = opool.tile([_OLOC, _B], f32)
            for h in range(2):
                sl = slice(h * _NH, (h + 1) * _NH)
                nc.tensor.matmul(out=outp[:, sl], lhsT=st5_t[:], rhs=lxT_t[:, sl],
                                 start=True, stop=False, skip_group_check=True)

            bf_idx = 0
            dr_idx = 0
            pair_tile = [None]

            for bi in range(_NBLK):
                o, g = divmod(bi, 16)
                f8 = g in _F8_GS

                # ---------- h1 ----------
                h1_t = h1pool.tile([128, _B], bf16)
                he = h1_eng[bi]
                xg = xrep_ts[g // 2][:, (g % 2) * _B:(g % 2 + 1) * _B]
                if he == "act":
                    nc.scalar.activation(
                        h1_t[:], xg, AF.Lrelu,
                        bias=b1c_t[:, bi:bi + 1], scale=w1c_t[:, bi:bi + 1],
                        alpha=_ALPHA)
                else:  # DVE 3-instr bf16 path (two-AP-scalar fused affine)
                    t2_t = upool.tile([128, _B], bf16)
                    nc.vector.tensor_scalar(
                        out=t2_t[:], in0=xg, scalar1=w1c_t[:, bi:bi + 1],
                        scalar2=b1c_t[:, bi:bi + 1], op0=ALU.mult, op1=ALU.add)
                    u_t = upool.tile([128, _B], bf16)
                    nc.vector.tensor_scalar(
                        out=u_t[:], in0=t2_t[:], scalar1=_ALPHA,
                        scalar2=None, op0=ALU.mult)
                    nc.vector.tensor_tensor(
                        out=h1_t[:], in0=t2_t[:], in1=u_t[:], op=ALU.max)

                # ---------- mm2 ----------
                z2 = zpool.tile([128, _B], f32)
                for h in range(2):
                    sl = slice(h * _NH, (h + 1) * _NH)
                    nc.tensor.matmul(
                        out=z2[:, sl],
                        lhsT=w2blk_ts[bi // 16][:, (bi % 16) * 128:(bi % 16 + 1) * 128],
                        rhs=h1_t[:, sl], start=True, stop=True)

                # ---------- z2 evac ----------
                if f8:
                    if g % 2 == 0 or pair_tile[0] is None:
                        pair_tile[0] = h2ppool.tile([128, 2, _B], fp8, name="h2pair")
                    h2slot = pair_tile[0][:, g % 2, :]
                else:
                    h2bf = h2bpool.tile([128, _B], bf16)
                    h2slot = h2bf[:]
                if z2_eng[bi] == "act":
                    nc.scalar.activation(h2slot, z2[:], AF.Lrelu,
                                         bias=vb2_t[:, bi:bi + 1],
                                         scale=1.0, alpha=_ALPHA)
                else:  # DVE extract (bias) + Pool leaky finish
                    tt_ = upool.tile([128, _B], bf16)
                    nc.vector.tensor_scalar(
                        out=tt_[:], in0=z2[:], scalar1=vb2_t[:, bi:bi + 1],
                        scalar2=1.0, op0=ALU.add, op1=ALU.mult)
                    nc.gpsimd.scalar_tensor_tensor(
                        out=h2slot, in0=tt_[:], scalar=_ALPHA,
                        in1=tt_[:], op0=ALU.mult, op1=ALU.max)

                # ---------- mm4 ----------
                last = bi == _NBLK - 1
                if f8:
                    if g % 2 == 1:
                        pt = pair_tile[0]
                        for h in range(2):
                            sl = slice(h * _NH, (h + 1) * _NH)
                            nc.tensor.matmul(
                                out=outp[:, sl],
                                lhsT=st4d_t[:, :, dr_idx * _OLOC:(dr_idx + 1) * _OLOC],
                                rhs=pt[:, :, sl],
                                start=False, stop=last and h == 1,
                                perf_mode=PM.DoubleRow,
                                skip_group_check=True)
                        dr_idx += 1
                else:
                    for h in range(2):
                        sl = slice(h * _NH, (h + 1) * _NH)
                        nc.tensor.matmul(
                            out=outp[:, sl],
                            lhsT=st4b_t[:, bf_idx * _OLOC:(bf_idx + 1) * _OLOC],
                            rhs=h2bf[:, sl],
                            start=False, stop=last and h == 1,
                            skip_group_check=True)
                    bf_idx += 1

            outs = cpool.tile([_OLOC, _B], f32)
            for h in range(2):
                sl = slice(h * _NH, (h + 1) * _NH)
                nc.vector.tensor_copy(outs[:, sl], outp[:, sl])
                nc.sync.dma_start(out=out_d[:, sl], in_=outs[:, sl])

    nc.finalize()
    return nc


def _prepare_inputs(x, W1, b1, W2, b2, W3, b3, layer_w, bias_w):
    f = np.float32
    bfd = ml_dtypes.bfloat16
    f8d = ml_dtypes.float8_e4m3fn
    nf8 = len(_F8_GS)

    x = np.asarray(x, f)
    xT = np.ascontiguousarray(x.T)                       # [I, B]
    xt65 = np.concatenate([xT, np.ones((1, _B), f)], 0)  # [65, B]
    # xrep[(32j+h), g*B+b] = x[g*4+j, b]
    xrep = np.repeat(xT.reshape(16, 4, 1, _B), _H, axis=2)   # [16,4,H,B]
    xrep = np.ascontiguousarray(
        xrep.transpose(1, 2, 0, 3).reshape(128, 16 * _B))

    v = np.asarray(layer_w, f)[:, :, None] * np.asarray(W3, f)   # [O,I,H]
    av = np.abs(v)
    sgn = np.where(v >= 0, 1.0, -1.0).astype(f)
    W1 = np.asarray(W1, f); b1 = np.asarray(b1, f)
    W2 = np.asarray(W2, f); b2 = np.asarray(b2, f)
    layer_w = np.asarray(layer_w, f); bias_w = np.asarray(bias_w, f)
    b3 = np.asarray(b3, f)

    in_maps = []
    for c in range(_NCORES):
        osl = slice(c * _OLOC, (c + 1) * _OLOC)
        W1c, b1c, W2c, b2c = W1[osl], b1[osl], W2[osl], b2[osl]
        avc, sgc = av[osl], sgn[osl]
        lwc, bwc, b3c = layer_w[osl], bias_w[osl], b3[osl]

        w2blk = np.zeros((128, _NBLK, 128), f)
        vb2 = np.zeros((128, _NBLK), f)
        w1cc = np.zeros((128, _NBLK), f)
        b1cc = np.zeros((128, _NBLK), f)
        st4b_list, st4d_list = [], []
        pend_dr = None

        for bi in range(_NBLK):
            o, g = divmod(bi, 16)
            f8 = g in _F8_GS
            sc = _F8_SCALE if f8 else 1.0
            edges = [g * 4 + j for j in range(4)]
            for j, i in enumerate(edges):
                w2blk[j * _H:(j + 1) * _H, bi, j * _H:(j + 1) * _H] = \
                    (sc * avc[o, i][:, None] * W2c[o, i]).T
                vb2[j * _H:(j + 1) * _H, bi] = sc * avc[o, i] * b2c[o, i]
                w1cc[j * _H:(j + 1) * _H, bi] = W1c[o, i]
                b1cc[j * _H:(j + 1) * _H, bi] = b1c[o, i]
            col = np.zeros((128, _OLOC), f)
            for j, i in enumerate(edges):
                col[j * _H:(j + 1) * _H, o] = sgc[o, i]
            if f8:
                if g % 2 == 0:
                    pend_dr = col / _F8_SCALE
                else:
                    st4d_list.append((pend_dr, col / _F8_SCALE))
            else:
                st4b_list.append(col)

        st4b = np.concatenate(st4b_list, axis=1)
        ndr = len(st4d_list)
        st4d = np.zeros((128, 2, ndr * _OLOC), f)
        for t, (c0, c1) in enumerate(st4d_list):
            st4d[:, 0, t * _OLOC:(t + 1) * _OLOC] = c0
            st4d[:, 1, t * _OLOC:(t + 1) * _OLOC] = c1

        st5 = np.zeros((65, _OLOC), f)
        st5[:_I, :] = bwc.T
        st5[_I, :] = (lwc * b3c).sum(1)

        m = {
            "xt65": xt65.astype(bfd),
            "xrep": xrep.astype(bfd),
            "w2blk": np.ascontiguousarray(
                w2blk.reshape(128, _NBLK * 128)).astype(bfd),
            "w1c": w1cc, "b1c": b1cc, "vb2": vb2,
            "aw1c": _ALPHA * w1cc, "ab1c": _ALPHA * b1cc,
            "st4b": st4b.astype(bfd),
            "st5": st5.astype(bfd),
        }
        if ndr:
            m["st4d"] = np.ascontiguousarray(
                st4d.reshape(128, 2 * ndr * _OLOC)).astype(f8d)
        in_maps.append(m)
    return in_maps


def kernel(x, W1, b1, W2, b2, W3, b3, layer_w, bias_w):
    from concourse.bass_utils import run_bass_kernel_spmd

    if "nc" not in _CACHE:
        _CACHE["nc"] = _build_bass()
    nc = _CACHE["nc"]

    in_maps = _prepare_inputs(x, W1, b1, W2, b2, W3, b3, layer_w, bias_w)
    res = run_bass_kernel_spmd(nc, in_maps, list(range(_NCORES))).results

    out = np.empty((_B, _O), np.float32)
    for c in range(_NCORES):
        out[:, c * _OLOC:(c + 1) * _OLOC] = res[c]["out"].T
    return out
